# revision 28
# baseline (speedup 1.0000x reference)
"""Multi-head attention (B=2, S=2048, D=1024, H=16) on 8 trn2 NeuronCores.

Sharding: core c handles batch c//4 and head-group c%4 (4 heads, dh'=256
slice of the projection dims).  Each core computes its heads' Q/K/V
projections, transposed-layout attention (scores as [keys, q] so softmax
runs one pass per key chunk and A@V contracts keys on partitions), and a
partial output projection against its Wo column slice.  The host sums the
4 partials per batch and adds bo.

All data stays bf16 (fp8 was measured at 4-8% output error here: attention
output is a cancellation-heavy average, so per-element quantization error
lands flat on the output instead of averaging away).  On top of the bf16
baseline, the softmax exp+mask work - which saturated the Scalar engine at
~168us - is split across THREE engines per key chunk:
- path A (9/16 chunks): ACT exp -> bf16, then ONE DVE u32 AND against a
  0xFFFF/0x0000 mask;
- path B (5/16 chunks, kc%3==2): ONE DVE scalar_tensor_tensor
  (s + 16128) min maskv -> u16: f32->u16 conversion is RNE+saturating, so
  the u16 IS the bf16 bit pattern of exp(s)/2 (Schraudolph, 1.9% rms);
  maskv = {19968.0 keep, -1.0 masked (saturates to bits 0 = +0.0)};
- path G (2/16 chunks, kc in {4,12}): ACT exp, then the mask rides the
  otherwise-idle Pool engine as a multiply by a {1.0, 0.0} bf16 mask.
Scores ship pre-scaled by 128/ln2 (qp,kp each carry alpha=4.8045) so both
exp paths read the same PSUM; path A matches path B's +4.07% sawtooth mean
via its activation bias, and the shared scale cancels in the softmax
normalize.  The mask ships as u16 with the per-(kc,qt)-block form chosen
to match the chunk's path.

Other structure (from the tuned bf16 baseline):
- activations ship pre-transposed ([D, S]) so projections contract D on
  partitions with zero on-chip transposes;
- scores/AV run per head with K=64; two heads of a pair sit at SBUF
  partitions 0-63/64-127 so their matmuls row-pack into the PE;
- row sums come from a ones-column appended to V; they bounce through a
  DRAM scratch to a [128, 16] layout so the reciprocal runs at full
  partition parallelism, then broadcast back with a second DRAM hop;
- the normalize multiplies run on the Pool engine; the A@V PSUM drains and
  half the out-projection drains run on the ACT engine (its exp load
  dropped ~35% from the split) and the rest on the DVE;
- emission order is the schedule: one software-pipelined pass per query
  tile; AV matmuls trail their scores by one key chunk; k/v projections
  (first tile), next tile's q-projection, previous tile's normalization
  and out-projection are spliced into the key-chunk loops in <=1us pieces;
- DMA: the ~3MB critical head (wk/kx/wq/qx) is balanced ~1MB per queue
  across sync/scalar/gpsimd, the first mask arrives in per-4-chunk
  quarters, wo/bias loads are deferred into the loop.
"""

import os
import sys

for _p in ("/opt/trn_rl_repo",):
    if _p not in sys.path and os.path.isdir(_p):
        sys.path.insert(0, _p)

import ml_dtypes
import numpy as np

import concourse.bass as bass
import concourse.mybir as mybir
import concourse.tile as tile
from concourse.vector_clock import ScopedClock
from concourse.bass_utils import run_bass_kernel_spmd


def _ensure_axon_hooks_stub():
    """bass_utils imports antenv.axon_hooks when BASS_TRACE=1 under axon;
    this image lacks the module.  Provide a no-hook stub (tracing is then
    skipped gracefully) unless a real one is already installed."""
    try:
        import antenv.axon_hooks  # noqa: F401
    except ImportError:
        import types

        import antenv

        mod = types.ModuleType("antenv.axon_hooks")
        mod._hook = None
        mod.set_axon_ntff_profile_hook = lambda h: setattr(mod, "_hook", h)
        mod.get_axon_ntff_profile_hook = lambda: mod._hook
        sys.modules["antenv.axon_hooks"] = mod
        antenv.axon_hooks = mod


_ensure_axon_hooks_stub()

F32 = mybir.dt.float32
BF16 = mybir.dt.bfloat16
U8 = mybir.dt.uint8
U16 = mybir.dt.uint16
U32 = mybir.dt.uint32
EXP = mybir.ActivationFunctionType.Exp
MUL = mybir.AluOpType.mult
ADD = mybir.AluOpType.add
MIN = mybir.AluOpType.min
AND = mybir.AluOpType.bitwise_and

B, S, D, H, DH = 2, 2048, 1024, 16, 64
NCORES = 8
GH = 4            # heads per core
GD = GH * DH      # 256, dh' slice per core
P = 128
NDC = D // P      # 8 contraction chunks
NQT = 4           # 512-wide query tiles
QT = 512
NKC = S // P      # 16 key chunks
NTT = S // P      # 16 token tiles

# softmax scaling: qp,kp each carry ALPHA so the score PSUM equals
# s_true * 128/ln2 = bf16-bits-per-e-fold
ALPHA = 4.8044896
K16 = 16128.0                  # path-B bits bias: pm = exp(s)/2
A_SCALE = 0.0054152123         # ln2/128
A_BIAS = -0.6532618            # -ln2 + ln(1.0407) sawtooth-mean match
MKEEP_B = 0x469C               # bf16 19968.0
MMASK_B = 0xBF80               # bf16 -1.0
MKEEP_G = 0x3F80               # bf16 1.0
MMASK_G = 0x0000


def _form(qt: int, kc: int) -> str:
    """exp+mask path per key chunk (must not depend on pair: the mask
    block form is shared between pairs).  Half the chunks ride path B:
    per-chunk engine cost is 1.38us (DVE only) vs 2.39us for path A
    (ACT exp + DVE AND), so this split leaves the ACT stream ~35%
    loaded and exps fire the moment their scores land."""
    return "B" if kc % 2 == 0 else "A"


# ---------------------------------------------------------------------------
# Walrus-compat shims: this neuronxcc build encodes at most ONE sync wait per
# instruction; Tile's wait assigner emits more.  Hoist overflow waits onto
# injected same-engine NOPs placed immediately before the instruction.
# ---------------------------------------------------------------------------
class _TC(tile.TileContext):
    def _drain_and_barrier(self, tick_clock, wait_clock):
        carrier = self.nc.sync.nop(nofuse=True, hint="tail_waits")
        wait_clock.add_sem_waits(
            carrier.ins, ScopedClock({None: tick_clock.global_clock})
        )
        si = carrier.ins.sync_info
        evs = list(si.on_wait) if si is not None else []
        carrier.ins.sync_info = mybir.SyncInfo(on_wait=evs[:1], on_update=[])
        for k in range(1, len(evs)):
            w = self.nc.sync.nop(nofuse=True, hint=f"tail_wait_{k}")
            w.ins.sync_info = mybir.SyncInfo(on_wait=[evs[k]], on_update=[])
        self.nc.sync.drain()
        self.nc.all_engine_barrier()
        assert self.sems is not None
        popped = self.nc._tile_sem_poison_stack.pop()
        assert popped is self._sem_poison
        self.nc.clear_and_free_semaphores(list(self.sems.allocated().values()))
        self.nc.all_engine_barrier()


def _split_excess_waits(nc: bass.Bass) -> int:
    n_split = 0
    uid = 0
    for f in nc.m.functions:
        for bb in f.blocks:
            new_insts = []
            for inst in bb.instructions:
                si = inst.sync_info
                waits = list(si.on_wait) if si is not None else []
                if len(waits) > 1:
                    for ev in waits[:-1]:
                        nop = mybir.InstNoOp(
                            name=f"I-waitsplit-{uid}", ins=[], outs=[]
                        )
                        uid += 1
                        nop.engine = inst.engine
                        nop.bass_nofuse = True
                        nop.sync_info = mybir.SyncInfo(
                            on_wait=[ev], on_update=[]
                        )
                        new_insts.append(nop)
                        n_split += 1
                    inst.sync_info = mybir.SyncInfo(
                        on_wait=waits[-1:], on_update=list(si.on_update)
                    )
                new_insts.append(inst)
            bb.instructions = new_insts
    return n_split


# ---------------------------------------------------------------------------
# Device kernel (identical on all 8 cores; only the input data differs)
# ---------------------------------------------------------------------------
def _build_nc(zero_bias: bool) -> bass.Bass:
    nc = bass.Bass("TRN2", target_bir_lowering=False)

    qT = nc.dram_tensor("qT", [D, S], BF16, kind="ExternalInput")
    kT = nc.dram_tensor("kT", [D, S], BF16, kind="ExternalInput")
    vT = nc.dram_tensor("vT", [D, S], BF16, kind="ExternalInput")
    # mask u16, per-(kc,qt)-block form matching the chunk's exp path
    maskT = nc.dram_tensor("maskT", [S, S], U16, kind="ExternalInput")
    # weights pre-arranged on the host to [P, NDC*GD] / [P, 2*D] lines
    wqT = nc.dram_tensor("wqT", [P, NDC * GD], BF16, kind="ExternalInput")
    wkT = nc.dram_tensor("wkT", [P, NDC * GD], BF16, kind="ExternalInput")
    wvT = nc.dram_tensor("wvT", [P, NDC * GD], BF16, kind="ExternalInput")
    bq = nc.dram_tensor("bq", [GD], F32, kind="ExternalInput")
    bk = nc.dram_tensor("bk", [GD], F32, kind="ExternalInput")
    bv = nc.dram_tensor("bv", [GD], F32, kind="ExternalInput")
    woT = nc.dram_tensor("woT", [P, 2 * D], BF16, kind="ExternalInput")
    y = nc.dram_tensor("y", [S, D], BF16, kind="ExternalOutput")

    with _TC(nc) as tc:
        with (
            tc.tile_pool(name="persist", bufs=1) as pp,
            tc.tile_pool(name="dram", bufs=4, space="DRAM") as dr,
        ):
            # ---- persistent SBUF state ----
            wq_s = pp.tile([P, NDC, GD], BF16)
            wk_s = pp.tile([P, NDC, GD], BF16)
            wv_s = pp.tile([P, NDC, GD], BF16)
            nc.scalar.dma_start(
                wk_s[:, 0:4, :],
                wkT[:].rearrange("p (c m) -> p c m", c=NDC)[:, 0:4, :],
            )
            bq_s = pp.tile([P, 2], F32)
            bk_s = pp.tile([P, 2], F32)
            nc.sync.dma_start(bk_s[:], bk[:].rearrange("(c p) -> p c", p=P))

            qpT = pp.tile([P, 2, S], BF16)   # [dh' within pair-chunk, pair, tok]
            kpT = pp.tile([P, 2, S], BF16)
            vp_aug = pp.tile([P, NKC, GH, DH + 1], BF16)
            concatT = pp.tile([P, 2, S], BF16)
            maskf2 = pp.tile([P, 2, NKC, QT], U16)
            ebias = pp.tile([P, 1], F32)

            nc.vector.memset(ebias[:], A_BIAS)
            nc.vector.memset(vp_aug[:, :, :, DH], 1.0)

            # touch EXP once immediately so the ~2.7us activation-table load
            # happens during the input-DMA head, not before the first real exp
            warm = pp.tile([1, 2], F32)
            nc.vector.memset(warm[:], 0.0)
            nc.scalar.activation(warm[0:1, 0:1], warm[0:1, 1:2], EXP)

            with (
                tc.tile_pool(name="xa", bufs=2) as xa,
                tc.tile_pool(name="eb", bufs=6) as eb,
                tc.tile_pool(name="nrm", bufs=2) as nrm,
                tc.tile_pool(name="yc", bufs=4) as yc,
                tc.tile_pool(name="psA", bufs=2, space="PSUM") as psA,
                tc.tile_pool(name="psS", bufs=4, space="PSUM") as psS,
                tc.tile_pool(name="psACC", bufs=2, space="PSUM") as psACC,
            ):
                ysb_tiles = {}

                def _cproj_tt(tt, last, nh_only=None):
                    """partial out-projection for token tile tt; emitted in
                    nh-halves (2 matmuls + a PSUM drain each); the drains
                    split between the DVE (nh0) and ACT (nh1) engines"""
                    nhs = (0, 1) if nh_only is None else (nh_only,)
                    if nhs[0] == 0:
                        ysb_tiles[tt] = yc.tile(
                            [P, D], BF16, tag="ysb", name=f"ysb{tt}"
                        )
                    y_sb = ysb_tiles[tt]
                    for nh in nhs:
                        yp = psA.tile([P, QT], F32, tag="proj", name="yp")
                        for pc in range(2):
                            nc.tensor.matmul(
                                yp[:],
                                concatT[:, pc, tt * P : (tt + 1) * P],
                                woT_s[:, pc, nh * QT : (nh + 1) * QT],
                                start=(pc == 0),
                                stop=(pc == 1),
                            )
                        if last and nh == 0:
                            nc.vector.tensor_copy(
                                y_sb[:, nh * QT : (nh + 1) * QT], yp[:]
                            )
                        else:
                            nc.scalar.copy(
                                y_sb[:, nh * QT : (nh + 1) * QT], yp[:]
                            )
                    if nhs[-1] == 1:
                        del ysb_tiles[tt]
                        if last:
                            # tail: spread the final y stores across queues
                            e0, e1 = (
                                (nc.sync, nc.scalar)
                                if tt % 2 == 0
                                else (nc.gpsimd, nc.sync)
                            )
                            rows = slice(tt * P, (tt + 1) * P)
                            e0.dma_start(y[rows, 0:QT], y_sb[:, 0:QT])
                            e1.dma_start(y[rows, QT:D], y_sb[:, QT:D])
                        else:
                            nc.sync.dma_start(
                                y[tt * P : (tt + 1) * P, :], y_sb[:]
                            )

                def _cproj(qn, last):
                    for tt in range(4 * qn, 4 * qn + 4):
                        _cproj_tt(tt, last)

                qx_tiles = {}

                def _qproj_dma(qn, engs=None):
                    qs = slice(qn * QT, (qn + 1) * QT)
                    x_t = xa.tile([P, NDC, QT], BF16, tag="x", name=f"xq{qn}")
                    qsrc = qT[:].rearrange("(c p) t -> p c t", p=P)[:, :, qs]
                    engs = engs or (nc.sync, nc.sync)
                    engs[0].dma_start(x_t[:, 0:4, :], qsrc[:, 0:4, :])
                    engs[1].dma_start(x_t[:, 4:8, :], qsrc[:, 4:8, :])
                    qx_tiles[qn] = x_t

                # the 8-matmul accumulation chains of the q/k projections are
                # emitted in dc-halves so a single insertion into the key-chunk
                # loop never stalls the exp stream by more than ~0.5us
                proj_ps = {}

                def _qproj_mm(qn, pc, half=None):
                    qs = slice(qn * QT, (qn + 1) * QT)
                    x_t = qx_tiles[qn]
                    halves = (0, 1) if half is None else (half,)
                    if halves[0] == 0:
                        ps = psA.tile([P, QT], F32, tag="proj", name="psq")
                        proj_ps[("q", qn, pc)] = ps
                    else:
                        ps = proj_ps.pop(("q", qn, pc))
                    for h in halves:
                        for dc in range(4 * h, 4 * h + 4):
                            nc.tensor.matmul(
                                ps[:],
                                wq_s[:, dc, pc * P : (pc + 1) * P],
                                x_t[:, dc, :],
                                start=(dc == 0),
                                stop=(dc == NDC - 1),
                            )
                    if halves[-1] == 1:
                        if zero_bias:
                            nc.scalar.copy(qpT[:, pc, qs], ps[:])
                        else:
                            nc.vector.tensor_scalar_add(
                                qpT[:, pc, qs], ps[:], bq_s[:, pc : pc + 1]
                            )

                def _qproj(qn):
                    _qproj_dma(qn)
                    _qproj_mm(qn, 0)
                    _qproj_mm(qn, 1)

                kx_tiles = {}

                def _kproj_dma(qn, engs=None):
                    x_t = xa.tile(
                        [P, NDC, QT], BF16, tag="kx", bufs=3, name=f"xk{qn}"
                    )
                    ksrc = kT[:].rearrange("(c p) t -> p c t", p=P)[
                        :, :, qn * QT : (qn + 1) * QT
                    ]
                    engs = engs or (nc.sync, nc.sync)
                    engs[0].dma_start(x_t[:, 0:4, :], ksrc[:, 0:4, :])
                    engs[1].dma_start(x_t[:, 4:8, :], ksrc[:, 4:8, :])
                    kx_tiles[qn] = x_t

                def _kproj_mm(qn, pc, half=None):
                    x_t = kx_tiles[qn]
                    halves = (0, 1) if half is None else (half,)
                    if halves[0] == 0:
                        ps = psA.tile([P, QT], F32, tag="proj", name="psk")
                        proj_ps[("k", qn, pc)] = ps
                    else:
                        ps = proj_ps.pop(("k", qn, pc))
                    for h in halves:
                        for dc in range(4 * h, 4 * h + 4):
                            nc.tensor.matmul(
                                ps[:],
                                wk_s[:, dc, pc * P : (pc + 1) * P],
                                x_t[:, dc, :],
                                start=(dc == 0),
                                stop=(dc == NDC - 1),
                            )
                    if halves[-1] == 1:
                        if zero_bias:
                            nc.scalar.copy(
                                kpT[:, pc, qn * QT : (qn + 1) * QT], ps[:]
                            )
                        else:
                            nc.vector.tensor_scalar_add(
                                kpT[:, pc, qn * QT : (qn + 1) * QT],
                                ps[:],
                                bk_s[:, pc : pc + 1],
                            )

                def _kproj(qn):
                    _kproj_dma(qn)
                    _kproj_mm(qn, 0)
                    _kproj_mm(qn, 1)

                vx_tiles = {}

                def _vx_dma(j, engs=None):
                    v_t = xa.tile(
                        [P, NDC, QT], BF16, tag="vx", bufs=2, name=f"xv{j}"
                    )
                    vsrc = vT[:].rearrange("(c p) t -> p c t", p=P)[
                        :, :, j * QT : (j + 1) * QT
                    ]
                    engs = engs or (nc.sync, nc.sync)
                    engs[0].dma_start(v_t[:, 0:4, :], vsrc[:, 0:4, :])
                    engs[1].dma_start(v_t[:, 4:8, :], vsrc[:, 4:8, :])
                    vx_tiles[j] = v_t

                # head: the ~3MB of critical first inputs (wk/kx/wq/qx) is
                # balanced ~1MB per queue across sync/scalar/gpsimd, followed
                # by the v inputs, with the mask in per-4-chunk quarters
                # critical first inputs balanced across the three DMA issue
                # queues (sync/scalar/gpsimd), ~1MB each, strictly ahead of
                # the v-path and mask bytes
                # queue plan (per-queue order IS arrival order):
                #   sync:   kx[0:4] kx[4:8] wv[0:4] vx0-lo
                #   scalar: wk[0:4](persist) wk[4:8] qx[4:8] wv[4:8] mask...
                #   gpsimd: wq[0:4] qx[0:4] wq[4:8] bq vx0-hi bv mask...
                _kproj_dma(0, engs=(nc.sync, nc.sync))
                wkr = wkT[:].rearrange("p (c m) -> p c m", c=NDC)
                nc.scalar.dma_start(wk_s[:, 4:8, :], wkr[:, 4:8, :])
                wqr = wqT[:].rearrange("p (c m) -> p c m", c=NDC)
                nc.gpsimd.dma_start(wq_s[:, 0:4, :], wqr[:, 0:4, :])
                qs0 = qT[:].rearrange("(c p) t -> p c t", p=P)[:, :, 0:QT]
                x_q0 = xa.tile([P, NDC, QT], BF16, tag="x", name="xq0")
                nc.gpsimd.dma_start(x_q0[:, 0:4, :], qs0[:, 0:4, :])
                nc.scalar.dma_start(x_q0[:, 4:8, :], qs0[:, 4:8, :])
                nc.gpsimd.dma_start(wq_s[:, 4:8, :], wqr[:, 4:8, :])
                qx_tiles[0] = x_q0
                nc.gpsimd.dma_start(bq_s[:], bq[:].rearrange("(c p) -> p c", p=P))
                _kproj_mm(0, 0, half=0)
                _qproj_mm(0, 0, half=0)
                _kproj_mm(0, 0, half=1)
                _qproj_mm(0, 0, half=1)
                wvr = wvT[:].rearrange("p (c m) -> p c m", c=NDC)
                nc.sync.dma_start(wv_s[:, 0:4, :], wvr[:, 0:4, :])
                nc.scalar.dma_start(wv_s[:, 4:8, :], wvr[:, 4:8, :])
                _vx_dma(0, engs=(nc.sync, nc.gpsimd))
                bv_b = pp.tile([P, GD], F32)
                nc.gpsimd.dma_start(bv_b[:], bv[:][None, :].to_broadcast((P, GD)))
                mr0 = maskT[:, 0:QT].rearrange("(c p) t -> p c t", p=P)
                for mq, meng in zip(
                    range(4), (nc.gpsimd, nc.scalar, nc.gpsimd, nc.scalar)
                ):
                    meng.dma_start(
                        maskf2[:, 0, 4 * mq : 4 * mq + 4, :],
                        mr0[:, 4 * mq : 4 * mq + 4, :],
                    )
                woT_s = pp.tile([P, 2, D], BF16)

                # ---- per query tile: q-proj, attention, partial out-proj --
                norm_state = {}

                def _norm_a(qn, eng=None):
                    av_sb, d1 = norm_state[qn]
                    s128 = nrm.tile([128, 16], BF16, tag="s128")
                    (eng or nc.sync).dma_start(s128[:], d1[:])
                    norm_state[qn] = (av_sb, s128)

                def _norm_b(qn, eng=None):
                    av_sb, s128 = norm_state[qn]
                    r128 = nrm.tile([128, 16], BF16, tag="r128")
                    with nc.allow_low_precision(
                        reason="softmax denominators carry bf16 noise already"
                    ):
                        nc.vector.reciprocal(r128[:], s128[:])
                    d2 = dr.tile([128, 16], BF16, tag="d2")
                    (eng or nc.sync).dma_start(d2[:], r128[:])
                    rb4 = nrm.tile([64, 4, QT], BF16, tag="rb4")
                    d2v = d2[:].rearrange("p c -> (p c)").rearrange(
                        "(h q) -> h q", h=4
                    )
                    for hh in range(4):
                        heng = (eng or nc.sync) if hh % 2 == 0 else nc.sync
                        heng.dma_start(
                            rb4[:, hh : hh + 1, :],
                            d2v[hh : hh + 1][None, :, :].to_broadcast(
                                (64, 1, QT)
                            ),
                        )
                    norm_state[qn] = (av_sb, rb4)

                def _norm_c(qn):
                    # normalize multiplies ride the Pool engine
                    av_sb, rb4 = norm_state[qn]
                    qs = slice(qn * QT, (qn + 1) * QT)
                    for hh in range(4):
                        nc.gpsimd.tensor_tensor(
                            concatT[64 * (hh % 2) : 64 * (hh % 2) + 64, hh // 2, qs],
                            av_sb[0:64, hh, :],
                            rb4[:, hh, :],
                            MUL,
                        )

                # pair-granular normalize for the last tile: pair0's
                # reciprocal pipeline overlaps pair1's key-chunk loop, so
                # only pair1's DRAM bounce sits in the tail
                half_state = {}

                def _normh_a(qn, pr, eng):
                    _av_sb, d1 = norm_state[qn]
                    s64 = nrm.tile([64, 16], BF16, tag=f"s64_{pr}")
                    eng.dma_start(s64[:], d1[64 * pr : 64 * pr + 64, :])
                    half_state[(qn, pr)] = s64

                def _normh_b(qn, pr, eng):
                    s64 = half_state[(qn, pr)]
                    r64 = nrm.tile([64, 16], BF16, tag=f"r64_{pr}")
                    with nc.allow_low_precision(
                        reason="softmax denominators carry bf16 noise already"
                    ):
                        nc.vector.reciprocal(r64[:], s64[:])
                    d2 = dr.tile([64, 16], BF16, tag=f"dh2_{pr}")
                    eng.dma_start(d2[:], r64[:])
                    if (qn, "rb4") not in half_state:
                        half_state[(qn, "rb4")] = nrm.tile(
                            [64, 4, QT], BF16, tag="rb4", name="rb4h"
                        )
                    rb4 = half_state[(qn, "rb4")]
                    d2v = d2[:].rearrange("p c -> (p c)").rearrange(
                        "(h q) -> h q", h=2
                    )
                    for i, hh in enumerate((2 * pr, 2 * pr + 1)):
                        heng = eng if i % 2 == 0 else nc.sync
                        heng.dma_start(
                            rb4[:, hh : hh + 1, :],
                            d2v[i : i + 1][None, :, :].to_broadcast(
                                (64, 1, QT)
                            ),
                        )

                def _normh_c(qn, pr):
                    av_sb, _d1 = norm_state[qn]
                    rb4 = half_state[(qn, "rb4")]
                    qs = slice(qn * QT, (qn + 1) * QT)
                    for hh in (2 * pr, 2 * pr + 1):
                        nc.gpsimd.tensor_tensor(
                            concatT[64 * (hh % 2) : 64 * (hh % 2) + 64, hh // 2, qs],
                            av_sb[0:64, hh, :],
                            rb4[:, hh, :],
                            MUL,
                        )

                for qt in range(NQT):
                    qsl = slice(qt * QT, (qt + 1) * QT)
                    mbuf = qt % 2

                    av_sb = nrm.tile([65, 4, QT], BF16, tag="av_sb")
                    d1 = dr.tile([128, 16], BF16, tag="d1")
                    norm_state[qt] = (av_sb, d1)
                    for pair in range(2):
                        avs2 = psACC.tile(
                            [P, 2, QT], F32, tag="acc", bufs=1, name="av2"
                        )

                        def _av(kcd, pm, pair=pair, avs2=avs2):
                            for h2 in range(2):
                                nc.tensor.matmul(
                                    avs2[0 : DH + 1, h2, :],
                                    vp_aug[:, kcd, 2 * pair + h2, :],
                                    pm[:, h2, :],
                                    start=(kcd == 0),
                                    stop=(kcd == NKC - 1),
                                )

                        # software-pipelined: AV for chunk kc-2 is emitted
                        # after the scores of chunk kc, so the exp+mask of a
                        # chunk has two chunk-times to land before the PE
                        # needs its pm
                        pm_hist = {}
                        for kc in range(NKC):
                            # per-h2 single-bank score tiles: 4-deep psS so
                            # the PE runs 2 chunks ahead of the exp stream
                            scs = []
                            for h2 in range(2):
                                lo = 64 * h2
                                sch = psS.tile([P, QT], F32, tag="sc")
                                nc.tensor.matmul(
                                    sch[:],
                                    kpT[lo : lo + 64, pair, kc * P : (kc + 1) * P],
                                    qpT[lo : lo + 64, pair, qsl],
                                )
                                scs.append(sch)
                            if qt == 0 and pair == 0:
                                # k/v projections ride the first pass
                                for f in {
                                    0: [lambda: _kproj_dma(1)],
                                    1: [lambda: _vx_dma(1),
                                        lambda: _kproj_mm(0, 1)],
                                    2: [lambda: _kproj_mm(1, 0)],
                                    3: [lambda: _kproj_dma(2),
                                        lambda: _qproj_mm(0, 1)],
                                    4: [lambda: _kproj_mm(1, 1)],
                                    5: [lambda: _vx_dma(2)],
                                    6: [lambda: _kproj_mm(2, 0)],
                                    7: [lambda: _kproj_dma(3),
                                        lambda: _kproj_mm(2, 1)],
                                    9: [lambda: _vx_dma(3)],
                                    10: [lambda: _kproj_mm(3, 0)],
                                    11: [lambda: _kproj_mm(3, 1)],
                                }.get(kc, []):
                                    f()
                                v_t = vx_tiles[kc // 4]
                                ps = psA.tile([P, GD], F32, tag="proj", name="psv")
                                for dc in range(NDC):
                                    nc.tensor.matmul(
                                        ps[:],
                                        v_t[:, dc, (kc % 4) * P : (kc % 4 + 1) * P],
                                        wv_s[:, dc, :],
                                        start=(dc == 0),
                                        stop=(dc == NDC - 1),
                                    )
                                if zero_bias:
                                    nc.scalar.copy(
                                        vp_aug[:, kc, :, 0:DH],
                                        ps[:].rearrange(
                                            "p (h d) -> p h d", h=GH
                                        ),
                                    )
                                else:
                                    nc.vector.tensor_tensor(
                                        vp_aug[:, kc, :, 0:DH],
                                        ps[:].rearrange(
                                            "p (h d) -> p h d", h=GH
                                        ),
                                        bv_b[:].rearrange(
                                            "p (h d) -> p h d", h=GH
                                        ),
                                        ADD,
                                    )
                            if pair == 0 and qt + 1 < NQT and (
                                kc == (12 if qt == 0 else 0)
                            ):
                                _qproj_dma(qt + 1)
                            if pair == 0 and kc == 8 and qt + 1 < NQT:
                                # next tile's mask prefetch (deferred off the
                                # critical input head)
                                nsl = slice((qt + 1) * QT, (qt + 2) * QT)
                                nc.gpsimd.dma_start(
                                    maskf2[:, (qt + 1) % 2, :, :],
                                    maskT[:, nsl].rearrange(
                                        "(c p) t -> p c t", p=P
                                    ),
                                )
                            if pair == 0 and qt > 0:
                                if kc == 0:
                                    _norm_a(qt - 1)
                                if kc == 8:
                                    _norm_b(qt - 1)
                                if kc == 5 and qt + 1 < NQT:
                                    _qproj_mm(qt + 1, 0, half=0)
                                if kc == 7 and qt + 1 < NQT:
                                    _qproj_mm(qt + 1, 0, half=1)
                                if kc == 10 and qt + 1 < NQT:
                                    _qproj_mm(qt + 1, 1, half=0)
                                if kc == 12 and qt + 1 < NQT:
                                    _qproj_mm(qt + 1, 1, half=1)
                            if pair == 1:
                                if qt == 0:
                                    if kc == 0:
                                        nc.sync.dma_start(
                                            woT_s[:],
                                            woT[:].rearrange(
                                                "p (c n) -> p c n", c=2
                                            ),
                                        )
                                    if kc == 2:
                                        _qproj_mm(1, 0, half=0)
                                    if kc == 3:
                                        _qproj_mm(1, 0, half=1)
                                    if kc == 5:
                                        _qproj_mm(1, 1, half=0)
                                    if kc == 6:
                                        _qproj_mm(1, 1, half=1)
                                if kc == 0 and qt > 0:
                                    _norm_c(qt - 1)
                                if qt == NQT - 1:
                                    if kc == 3:
                                        _normh_a(qt, 0, nc.scalar)
                                    if kc == 9:
                                        _normh_b(qt, 0, nc.scalar)
                                    if kc == 14:
                                        _normh_c(qt, 0)
                                if qt > 0 and kc in (2, 4, 6, 8, 9, 10, 12, 13):
                                    ih = (2, 4, 6, 8, 9, 10, 12, 13).index(kc)
                                    _cproj_tt(
                                        4 * (qt - 1) + ih // 2, False,
                                        nh_only=ih % 2,
                                    )
                            if kc >= 2:
                                _av(kc - 2, pm_hist.pop(kc - 2))
                            # exp + mask, path per chunk
                            pm = eb.tile([P, 2, QT], BF16, tag="pm", bufs=5)
                            fm = _form(qt, kc)
                            mop = maskf2[:, mbuf, kc, :]
                            if fm == "B":
                                for h2 in range(2):
                                    nc.vector.scalar_tensor_tensor(
                                        pm[:, h2, :].bitcast(U16),
                                        scs[h2][:],
                                        K16,
                                        mop.bitcast(BF16),
                                        ADD,
                                        MIN,
                                    )
                            else:
                                ex = eb.tile([P, 2, QT], BF16, tag="ex", bufs=3)
                                for h2 in range(2):
                                    nc.scalar.activation(
                                        ex[:, h2, :], scs[h2][:], EXP,
                                        bias=ebias[:], scale=A_SCALE,
                                    )
                                for h2 in range(2):
                                    nc.vector.tensor_tensor(
                                        pm[:, h2, :].bitcast(U32),
                                        ex[:, h2, :].bitcast(U32),
                                        mop.bitcast(U32),
                                        AND,
                                    )
                            pm_hist[kc] = pm
                        _av(NKC - 2, pm_hist.pop(NKC - 2))
                        _av(NKC - 1, pm_hist.pop(NKC - 1))
                        # drain av (incl. its sum row 64) out of PSUM on the
                        # ACT engine (its exp load dropped with the split)
                        d1v = d1[:].rearrange("p c -> (p c)").rearrange(
                            "(h q) -> h q", h=4
                        )
                        xeng = (
                            nc.scalar
                            if (qt == NQT - 1 and pair == 1)
                            else nc.sync
                        )
                        nc.scalar.copy(
                            av_sb[:, 2 * pair : 2 * pair + 2, :],
                            avs2[0 : DH + 1, :, :],
                        )
                        for h2 in range(2):
                            hh = 2 * pair + h2
                            xeng.dma_start(
                                d1v[hh : hh + 1, :], av_sb[64:65, hh, :]
                            )
                # tail: only pair1's half-normalize remains; short junk-MM
                # bridges keep the PE clock warm across its DMA bounces
                junkps = psACC.tile(
                    [P, 2, QT], F32, tag="acc", bufs=1, name="junkps"
                )

                def _junk(n):
                    for wi in range(n):
                        nc.tensor.matmul(
                            junkps[:, 0, 0:GD],
                            wq_s[:, wi % NDC, 0:P],
                            wq_s[:, (wi + 1) % NDC, :],
                            start=True,
                            stop=True,
                        )

                _normh_a(NQT - 1, 1, nc.scalar)
                _junk(16)
                _normh_b(NQT - 1, 1, nc.scalar)
                _junk(58)
                _normh_c(NQT - 1, 1)
                _cproj(NQT - 1, last=True)

    _split_excess_waits(nc)
    return nc


_NC_CACHE = {}
LAST_RESULTS = None  # test harness reads exec_time_ns off this


def kernel(q, k, v, mask, Wq, bq, Wk, bk, Wv, bv, Wo, bo):
    global LAST_RESULTS
    zb = not (
        np.any(np.asarray(bq)) or np.any(np.asarray(bk))
        or np.any(np.asarray(bv))
    )
    if zb not in _NC_CACHE:
        _NC_CACHE[zb] = _build_nc(zb)
    _NC = _NC_CACHE[zb]

    q = np.asarray(q, np.float32)
    k = np.asarray(k, np.float32)
    v = np.asarray(v, np.float32)

    bf = ml_dtypes.bfloat16
    qTb = [np.ascontiguousarray(q[b].T.astype(bf)) for b in range(B)]
    kTb = [np.ascontiguousarray(k[b].T.astype(bf)) for b in range(B)]
    vTb = [np.ascontiguousarray(v[b].T.astype(bf)) for b in range(B)]

    # mask u16: per-(kc, qt) block form matching the chunk's exp path
    m_keys_q = np.asarray(mask)[0, 0].T != 0   # [keys, q]
    maskT_u16 = np.empty((S, S), np.uint16)
    for kc in range(NKC):
        rows = slice(kc * P, (kc + 1) * P)
        for qt in range(NQT):
            cols = slice(qt * QT, (qt + 1) * QT)
            blk = m_keys_q[rows, cols]
            fm = _form(qt, kc)
            if fm == "B":
                maskT_u16[rows, cols] = np.where(blk, MKEEP_B, MMASK_B)
            elif fm == "G":
                maskT_u16[rows, cols] = np.where(blk, MKEEP_G, MMASK_G)
            else:
                maskT_u16[rows, cols] = np.where(
                    blk, np.uint16(0xFFFF), np.uint16(0)
                )

    Wq_ = np.asarray(Wq, np.float32) * ALPHA
    Wk_ = np.asarray(Wk, np.float32) * ALPHA
    Wv_ = np.asarray(Wv, np.float32)
    Wo_ = np.asarray(Wo, np.float32)

    def _warr(wT):  # [D, GD] -> [P, NDC*GD] per-partition-contiguous, bf16
        return np.ascontiguousarray(
            wT.reshape(NDC, P, GD)
            .transpose(1, 0, 2)
            .reshape(P, NDC * GD)
            .astype(bf)
        )

    in_maps = []
    for c in range(NCORES):
        b, g = divmod(c, NCORES // B)
        rows = slice(GD * g, GD * (g + 1))
        in_maps.append(
            {
                "qT": qTb[b],
                "kT": kTb[b],
                "vT": vTb[b],
                "maskT": maskT_u16,
                "wqT": _warr(Wq_[rows].T),
                "wkT": _warr(Wk_[rows].T),
                "wvT": _warr(Wv_[rows].T),
                "bq": np.ascontiguousarray(
                    np.asarray(bq, np.float32)[rows] * ALPHA
                ),
                "bk": np.ascontiguousarray(
                    np.asarray(bk, np.float32)[rows] * ALPHA
                ),
                "bv": np.ascontiguousarray(np.asarray(bv, np.float32)[rows]),
                "woT": np.ascontiguousarray(
                    Wo_[:, rows].T.reshape(2, P, D)
                    .transpose(1, 0, 2)
                    .reshape(P, 2 * D)
                    .astype(bf)
                ),
            }
        )

    res = run_bass_kernel_spmd(_NC, in_maps, core_ids=list(range(NCORES)))
    LAST_RESULTS = res

    ng = NCORES // B
    out = np.empty((B, S, D), np.float32)
    for b in range(B):
        acc = res.results[b * ng]["y"].astype(np.float32)
        for g in range(1, ng):
            acc += res.results[b * ng + g]["y"].astype(np.float32)
        out[b] = acc + np.asarray(bo, np.float32)
    return out


# revision 29
# speedup vs baseline: 1.0199x; 1.0199x over previous
"""Multi-head attention (B=2, S=2048, D=1024, H=16) on 8 trn2 NeuronCores.

Sharding: core c handles batch c//4 and head-group c%4 (4 heads, dh'=256
slice of the projection dims).  Each core computes its heads' Q/K/V
projections, transposed-layout attention (scores as [keys, q] so softmax
runs one pass per key chunk and A@V contracts keys on partitions), and a
partial output projection against its Wo column slice.  The host sums the
4 partials per batch and adds bo.

All data stays bf16 (fp8 was measured at 4-8% output error here: attention
output is a cancellation-heavy average, so per-element quantization error
lands flat on the output instead of averaging away).  On top of the bf16
baseline, the softmax exp+mask work - which saturated the Scalar engine at
~168us - is split across THREE engines per key chunk:
- path A (9/16 chunks): ACT exp -> bf16, then ONE DVE u32 AND against a
  0xFFFF/0x0000 mask;
- path B (5/16 chunks, kc%3==2): ONE DVE scalar_tensor_tensor
  (s + 16128) min maskv -> u16: f32->u16 conversion is RNE+saturating, so
  the u16 IS the bf16 bit pattern of exp(s)/2 (Schraudolph, 1.9% rms);
  maskv = {19968.0 keep, -1.0 masked (saturates to bits 0 = +0.0)};
- path G (2/16 chunks, kc in {4,12}): ACT exp, then the mask rides the
  otherwise-idle Pool engine as a multiply by a {1.0, 0.0} bf16 mask.
Scores ship pre-scaled by 128/ln2 (qp,kp each carry alpha=4.8045) so both
exp paths read the same PSUM; path A matches path B's +4.07% sawtooth mean
via its activation bias, and the shared scale cancels in the softmax
normalize.  The mask ships as u16 with the per-(kc,qt)-block form chosen
to match the chunk's path.

Other structure (from the tuned bf16 baseline):
- activations ship pre-transposed ([D, S]) so projections contract D on
  partitions with zero on-chip transposes;
- scores/AV run per head with K=64; two heads of a pair sit at SBUF
  partitions 0-63/64-127 so their matmuls row-pack into the PE;
- row sums come from a ones-column appended to V; they bounce through a
  DRAM scratch to a [128, 16] layout so the reciprocal runs at full
  partition parallelism, then broadcast back with a second DRAM hop;
- the normalize multiplies run on the Pool engine; the A@V PSUM drains and
  half the out-projection drains run on the ACT engine (its exp load
  dropped ~35% from the split) and the rest on the DVE;
- emission order is the schedule: one software-pipelined pass per query
  tile; AV matmuls trail their scores by one key chunk; k/v projections
  (first tile), next tile's q-projection, previous tile's normalization
  and out-projection are spliced into the key-chunk loops in <=1us pieces;
- DMA: the ~3MB critical head (wk/kx/wq/qx) is balanced ~1MB per queue
  across sync/scalar/gpsimd, the first mask arrives in per-4-chunk
  quarters, wo/bias loads are deferred into the loop.
"""

import os
import sys

for _p in ("/opt/trn_rl_repo",):
    if _p not in sys.path and os.path.isdir(_p):
        sys.path.insert(0, _p)

import ml_dtypes
import numpy as np

import concourse.bass as bass
import concourse.mybir as mybir
import concourse.tile as tile
from concourse.vector_clock import ScopedClock
from concourse.bass_utils import run_bass_kernel_spmd


def _ensure_axon_hooks_stub():
    """bass_utils imports antenv.axon_hooks when BASS_TRACE=1 under axon;
    this image lacks the module.  Provide a no-hook stub (tracing is then
    skipped gracefully) unless a real one is already installed."""
    try:
        import antenv.axon_hooks  # noqa: F401
    except ImportError:
        import types

        import antenv

        mod = types.ModuleType("antenv.axon_hooks")
        mod._hook = None
        mod.set_axon_ntff_profile_hook = lambda h: setattr(mod, "_hook", h)
        mod.get_axon_ntff_profile_hook = lambda: mod._hook
        sys.modules["antenv.axon_hooks"] = mod
        antenv.axon_hooks = mod


_ensure_axon_hooks_stub()

F32 = mybir.dt.float32
BF16 = mybir.dt.bfloat16
U8 = mybir.dt.uint8
U16 = mybir.dt.uint16
U32 = mybir.dt.uint32
EXP = mybir.ActivationFunctionType.Exp
MUL = mybir.AluOpType.mult
ADD = mybir.AluOpType.add
MIN = mybir.AluOpType.min
AND = mybir.AluOpType.bitwise_and

B, S, D, H, DH = 2, 2048, 1024, 16, 64
NCORES = 8
GH = 4            # heads per core
GD = GH * DH      # 256, dh' slice per core
P = 128
NDC = D // P      # 8 contraction chunks
NQT = 4           # 512-wide query tiles
QT = 512
NKC = S // P      # 16 key chunks
NTT = S // P      # 16 token tiles

# softmax scaling: qp,kp each carry ALPHA so the score PSUM equals
# s_true * 128/ln2 = bf16-bits-per-e-fold
ALPHA = 4.8044896
K16 = 16128.0                  # path-B bits bias: pm = exp(s)/2
A_SCALE = 0.0054152123         # ln2/128
A_BIAS = -0.6532618            # -ln2 + ln(1.0407) sawtooth-mean match
MKEEP_B = 0x469C               # bf16 19968.0
MMASK_B = 0xBF80               # bf16 -1.0
MKEEP_G = 0x3F80               # bf16 1.0
MMASK_G = 0x0000


def _form(qt: int, kc: int) -> str:
    """exp+mask path per key chunk (must not depend on pair: the mask
    block form is shared between pairs).  Half the chunks ride path B:
    per-chunk engine cost is 1.38us (DVE only) vs 2.39us for path A
    (ACT exp + DVE AND), so this split leaves the ACT stream ~35%
    loaded and exps fire the moment their scores land."""
    return "B" if kc % 2 == 0 else "A"


# ---------------------------------------------------------------------------
# Walrus-compat shims: this neuronxcc build encodes at most ONE sync wait per
# instruction; Tile's wait assigner emits more.  Hoist overflow waits onto
# injected same-engine NOPs placed immediately before the instruction.
# ---------------------------------------------------------------------------
class _TC(tile.TileContext):
    def _drain_and_barrier(self, tick_clock, wait_clock):
        carrier = self.nc.sync.nop(nofuse=True, hint="tail_waits")
        wait_clock.add_sem_waits(
            carrier.ins, ScopedClock({None: tick_clock.global_clock})
        )
        si = carrier.ins.sync_info
        evs = list(si.on_wait) if si is not None else []
        carrier.ins.sync_info = mybir.SyncInfo(on_wait=evs[:1], on_update=[])
        for k in range(1, len(evs)):
            w = self.nc.sync.nop(nofuse=True, hint=f"tail_wait_{k}")
            w.ins.sync_info = mybir.SyncInfo(on_wait=[evs[k]], on_update=[])
        self.nc.sync.drain()
        self.nc.all_engine_barrier()
        assert self.sems is not None
        popped = self.nc._tile_sem_poison_stack.pop()
        assert popped is self._sem_poison
        self.nc.clear_and_free_semaphores(list(self.sems.allocated().values()))
        self.nc.all_engine_barrier()


def _split_excess_waits(nc: bass.Bass) -> int:
    n_split = 0
    uid = 0
    for f in nc.m.functions:
        for bb in f.blocks:
            new_insts = []
            for inst in bb.instructions:
                si = inst.sync_info
                waits = list(si.on_wait) if si is not None else []
                if len(waits) > 1:
                    for ev in waits[:-1]:
                        nop = mybir.InstNoOp(
                            name=f"I-waitsplit-{uid}", ins=[], outs=[]
                        )
                        uid += 1
                        nop.engine = inst.engine
                        nop.bass_nofuse = True
                        nop.sync_info = mybir.SyncInfo(
                            on_wait=[ev], on_update=[]
                        )
                        new_insts.append(nop)
                        n_split += 1
                    inst.sync_info = mybir.SyncInfo(
                        on_wait=waits[-1:], on_update=list(si.on_update)
                    )
                new_insts.append(inst)
            bb.instructions = new_insts
    return n_split


# ---------------------------------------------------------------------------
# Device kernel (identical on all 8 cores; only the input data differs)
# ---------------------------------------------------------------------------
def _build_nc(zero_bias: bool) -> bass.Bass:
    nc = bass.Bass("TRN2", target_bir_lowering=False)

    qT = nc.dram_tensor("qT", [D, S], BF16, kind="ExternalInput")
    kT = nc.dram_tensor("kT", [D, S], BF16, kind="ExternalInput")
    vT = nc.dram_tensor("vT", [D, S], BF16, kind="ExternalInput")
    # mask u16, per-(kc,qt)-block form matching the chunk's exp path
    maskT = nc.dram_tensor("maskT", [S, S], U16, kind="ExternalInput")
    # weights pre-arranged on the host to [P, NDC*GD] / [P, 2*D] lines
    wqT = nc.dram_tensor("wqT", [P, NDC * GD], BF16, kind="ExternalInput")
    wkT = nc.dram_tensor("wkT", [P, NDC * GD], BF16, kind="ExternalInput")
    wvT = nc.dram_tensor("wvT", [P, NDC * GD], BF16, kind="ExternalInput")
    bq = nc.dram_tensor("bq", [GD], F32, kind="ExternalInput")
    bk = nc.dram_tensor("bk", [GD], F32, kind="ExternalInput")
    bv = nc.dram_tensor("bv", [GD], F32, kind="ExternalInput")
    woT = nc.dram_tensor("woT", [P, 2 * D], BF16, kind="ExternalInput")
    y = nc.dram_tensor("y", [S, D], BF16, kind="ExternalOutput")

    with _TC(nc) as tc:
        with (
            tc.tile_pool(name="persist", bufs=1) as pp,
            tc.tile_pool(name="dram", bufs=4, space="DRAM") as dr,
        ):
            # ---- persistent SBUF state ----
            wq_s = pp.tile([P, NDC, GD], BF16)
            wk_s = pp.tile([P, NDC, GD], BF16)
            wv_s = pp.tile([P, NDC, GD], BF16)
            nc.scalar.dma_start(
                wk_s[:, 0:4, :],
                wkT[:].rearrange("p (c m) -> p c m", c=NDC)[:, 0:4, :],
            )
            bq_s = pp.tile([P, 2], F32)
            bk_s = pp.tile([P, 2], F32)
            nc.sync.dma_start(bk_s[:], bk[:].rearrange("(c p) -> p c", p=P))

            qpT = pp.tile([P, 2, S], BF16)   # [dh' within pair-chunk, pair, tok]
            kpT = pp.tile([P, 2, S], BF16)
            vp_aug = pp.tile([P, NKC, GH, DH + 1], BF16)
            concatT = pp.tile([P, 2, S], BF16)
            maskf2 = pp.tile([P, 2, NKC, QT], U16)
            ebias = pp.tile([P, 1], F32)

            nc.vector.memset(ebias[:], A_BIAS)
            nc.vector.memset(vp_aug[:, :, :, DH], 1.0)

            # touch EXP once immediately so the ~2.7us activation-table load
            # happens during the input-DMA head, not before the first real exp
            warm = pp.tile([1, 2], F32)
            nc.vector.memset(warm[:], 0.0)
            nc.scalar.activation(warm[0:1, 0:1], warm[0:1, 1:2], EXP)

            with (
                tc.tile_pool(name="xa", bufs=2) as xa,
                tc.tile_pool(name="eb", bufs=6) as eb,
                tc.tile_pool(name="nrm", bufs=2) as nrm,
                tc.tile_pool(name="yc", bufs=4) as yc,
                tc.tile_pool(name="psA", bufs=2, space="PSUM") as psA,
                tc.tile_pool(name="psS", bufs=4, space="PSUM") as psS,
                tc.tile_pool(name="psACC", bufs=2, space="PSUM") as psACC,
            ):
                ysb_tiles = {}

                def _cproj_tt(tt, last, nh_only=None):
                    """partial out-projection for token tile tt; emitted in
                    nh-halves (2 matmuls + a PSUM drain each); the drains
                    split between the DVE (nh0) and ACT (nh1) engines"""
                    nhs = (0, 1) if nh_only is None else (nh_only,)
                    if nhs[0] == 0:
                        ysb_tiles[tt] = yc.tile(
                            [P, D], BF16, tag="ysb", name=f"ysb{tt}"
                        )
                    y_sb = ysb_tiles[tt]
                    for nh in nhs:
                        yp = psA.tile([P, QT], F32, tag="proj", name="yp")
                        for pc in range(2):
                            nc.tensor.matmul(
                                yp[:],
                                concatT[:, pc, tt * P : (tt + 1) * P],
                                woT_s[:, pc, nh * QT : (nh + 1) * QT],
                                start=(pc == 0),
                                stop=(pc == 1),
                            )
                        if last and nh == 0:
                            nc.vector.tensor_copy(
                                y_sb[:, nh * QT : (nh + 1) * QT], yp[:]
                            )
                        else:
                            nc.scalar.copy(
                                y_sb[:, nh * QT : (nh + 1) * QT], yp[:]
                            )
                    if nhs[-1] == 1:
                        del ysb_tiles[tt]
                        if last:
                            # tail: spread the final y stores across queues
                            e0, e1 = (
                                (nc.sync, nc.scalar)
                                if tt % 2 == 0
                                else (nc.gpsimd, nc.sync)
                            )
                            rows = slice(tt * P, (tt + 1) * P)
                            e0.dma_start(y[rows, 0:QT], y_sb[:, 0:QT])
                            e1.dma_start(y[rows, QT:D], y_sb[:, QT:D])
                        else:
                            nc.sync.dma_start(
                                y[tt * P : (tt + 1) * P, :], y_sb[:]
                            )

                def _cproj(qn, last):
                    for tt in range(4 * qn, 4 * qn + 4):
                        _cproj_tt(tt, last)

                qx_tiles = {}

                def _qproj_dma(qn, engs=None):
                    qs = slice(qn * QT, (qn + 1) * QT)
                    x_t = xa.tile([P, NDC, QT], BF16, tag="x", name=f"xq{qn}")
                    qsrc = qT[:].rearrange("(c p) t -> p c t", p=P)[:, :, qs]
                    engs = engs or (nc.sync, nc.sync)
                    engs[0].dma_start(x_t[:, 0:4, :], qsrc[:, 0:4, :])
                    engs[1].dma_start(x_t[:, 4:8, :], qsrc[:, 4:8, :])
                    qx_tiles[qn] = x_t

                # the 8-matmul accumulation chains of the q/k projections are
                # emitted in dc-halves so a single insertion into the key-chunk
                # loop never stalls the exp stream by more than ~0.5us
                proj_ps = {}

                def _qproj_mm(qn, pc, half=None):
                    qs = slice(qn * QT, (qn + 1) * QT)
                    x_t = qx_tiles[qn]
                    halves = (0, 1) if half is None else (half,)
                    if halves[0] == 0:
                        ps = psA.tile([P, QT], F32, tag="proj", name="psq")
                        proj_ps[("q", qn, pc)] = ps
                    else:
                        ps = proj_ps.pop(("q", qn, pc))
                    for h in halves:
                        for dc in range(4 * h, 4 * h + 4):
                            nc.tensor.matmul(
                                ps[:],
                                wq_s[:, dc, pc * P : (pc + 1) * P],
                                x_t[:, dc, :],
                                start=(dc == 0),
                                stop=(dc == NDC - 1),
                            )
                    if halves[-1] == 1:
                        if zero_bias:
                            nc.scalar.copy(qpT[:, pc, qs], ps[:])
                        else:
                            nc.vector.tensor_scalar_add(
                                qpT[:, pc, qs], ps[:], bq_s[:, pc : pc + 1]
                            )

                def _qproj(qn):
                    _qproj_dma(qn)
                    _qproj_mm(qn, 0)
                    _qproj_mm(qn, 1)

                kx_tiles = {}

                def _kproj_dma(qn, engs=None):
                    x_t = xa.tile(
                        [P, NDC, QT], BF16, tag="kx", bufs=3, name=f"xk{qn}"
                    )
                    ksrc = kT[:].rearrange("(c p) t -> p c t", p=P)[
                        :, :, qn * QT : (qn + 1) * QT
                    ]
                    engs = engs or (nc.sync, nc.sync)
                    engs[0].dma_start(x_t[:, 0:4, :], ksrc[:, 0:4, :])
                    engs[1].dma_start(x_t[:, 4:8, :], ksrc[:, 4:8, :])
                    kx_tiles[qn] = x_t

                def _kproj_mm(qn, pc, half=None):
                    x_t = kx_tiles[qn]
                    halves = (0, 1) if half is None else (half,)
                    if halves[0] == 0:
                        ps = psA.tile([P, QT], F32, tag="proj", name="psk")
                        proj_ps[("k", qn, pc)] = ps
                    else:
                        ps = proj_ps.pop(("k", qn, pc))
                    for h in halves:
                        for dc in range(4 * h, 4 * h + 4):
                            nc.tensor.matmul(
                                ps[:],
                                wk_s[:, dc, pc * P : (pc + 1) * P],
                                x_t[:, dc, :],
                                start=(dc == 0),
                                stop=(dc == NDC - 1),
                            )
                    if halves[-1] == 1:
                        if zero_bias:
                            nc.scalar.copy(
                                kpT[:, pc, qn * QT : (qn + 1) * QT], ps[:]
                            )
                        else:
                            nc.vector.tensor_scalar_add(
                                kpT[:, pc, qn * QT : (qn + 1) * QT],
                                ps[:],
                                bk_s[:, pc : pc + 1],
                            )

                def _kproj(qn):
                    _kproj_dma(qn)
                    _kproj_mm(qn, 0)
                    _kproj_mm(qn, 1)

                vx_tiles = {}

                def _vx_dma(j, engs=None):
                    v_t = xa.tile(
                        [P, NDC, QT], BF16, tag="vx", bufs=2, name=f"xv{j}"
                    )
                    vsrc = vT[:].rearrange("(c p) t -> p c t", p=P)[
                        :, :, j * QT : (j + 1) * QT
                    ]
                    engs = engs or (nc.sync, nc.sync)
                    engs[0].dma_start(v_t[:, 0:4, :], vsrc[:, 0:4, :])
                    engs[1].dma_start(v_t[:, 4:8, :], vsrc[:, 4:8, :])
                    vx_tiles[j] = v_t

                # head: the ~3MB of critical first inputs (wk/kx/wq/qx) is
                # balanced ~1MB per queue across sync/scalar/gpsimd, followed
                # by the v inputs, with the mask in per-4-chunk quarters
                # critical first inputs balanced across the three DMA issue
                # queues (sync/scalar/gpsimd), ~1MB each, strictly ahead of
                # the v-path and mask bytes
                # queue plan (per-queue order IS arrival order):
                #   sync:   kx[0:4] kx[4:8] wv[0:4] vx0-lo
                #   scalar: wk[0:4](persist) wk[4:8] qx[4:8] wv[4:8] mask...
                #   gpsimd: wq[0:4] qx[0:4] wq[4:8] bq vx0-hi bv mask...
                _kproj_dma(0, engs=(nc.sync, nc.sync))
                wkr = wkT[:].rearrange("p (c m) -> p c m", c=NDC)
                nc.scalar.dma_start(wk_s[:, 4:8, :], wkr[:, 4:8, :])
                wqr = wqT[:].rearrange("p (c m) -> p c m", c=NDC)
                nc.gpsimd.dma_start(wq_s[:, 0:4, :], wqr[:, 0:4, :])
                qs0 = qT[:].rearrange("(c p) t -> p c t", p=P)[:, :, 0:QT]
                x_q0 = xa.tile([P, NDC, QT], BF16, tag="x", name="xq0")
                nc.gpsimd.dma_start(x_q0[:, 0:4, :], qs0[:, 0:4, :])
                nc.scalar.dma_start(x_q0[:, 4:8, :], qs0[:, 4:8, :])
                nc.gpsimd.dma_start(wq_s[:, 4:8, :], wqr[:, 4:8, :])
                qx_tiles[0] = x_q0
                nc.gpsimd.dma_start(bq_s[:], bq[:].rearrange("(c p) -> p c", p=P))
                _kproj_mm(0, 0, half=0)
                _qproj_mm(0, 0, half=0)
                _kproj_mm(0, 0, half=1)
                _qproj_mm(0, 0, half=1)
                wvr = wvT[:].rearrange("p (c m) -> p c m", c=NDC)
                nc.sync.dma_start(wv_s[:, 0:4, :], wvr[:, 0:4, :])
                nc.scalar.dma_start(wv_s[:, 4:8, :], wvr[:, 4:8, :])
                _vx_dma(0, engs=(nc.sync, nc.gpsimd))
                bv_b = pp.tile([P, GD], F32)
                nc.gpsimd.dma_start(bv_b[:], bv[:][None, :].to_broadcast((P, GD)))
                mr0 = maskT[:, 0:QT].rearrange("(c p) t -> p c t", p=P)
                for mq, meng in zip(
                    range(4), (nc.gpsimd, nc.scalar, nc.gpsimd, nc.scalar)
                ):
                    meng.dma_start(
                        maskf2[:, 0, 4 * mq : 4 * mq + 4, :],
                        mr0[:, 4 * mq : 4 * mq + 4, :],
                    )
                woT_s = pp.tile([P, 2, D], BF16)

                # ---- per query tile: q-proj, attention, partial out-proj --
                norm_state = {}

                def _norm_a(qn, eng=None):
                    av_sb, d1 = norm_state[qn]
                    s128 = nrm.tile([128, 16], BF16, tag="s128")
                    (eng or nc.sync).dma_start(s128[:], d1[:])
                    norm_state[qn] = (av_sb, s128)

                def _norm_b(qn, eng=None):
                    av_sb, s128 = norm_state[qn]
                    r128 = nrm.tile([128, 16], BF16, tag="r128")
                    with nc.allow_low_precision(
                        reason="softmax denominators carry bf16 noise already"
                    ):
                        nc.vector.reciprocal(r128[:], s128[:])
                    d2 = dr.tile([128, 16], BF16, tag="d2")
                    (eng or nc.sync).dma_start(d2[:], r128[:])
                    rb4 = nrm.tile([64, 4, QT], BF16, tag="rb4")
                    d2v = d2[:].rearrange("p c -> (p c)").rearrange(
                        "(h q) -> h q", h=4
                    )
                    for hh in range(4):
                        heng = (eng or nc.sync) if hh % 2 == 0 else nc.sync
                        heng.dma_start(
                            rb4[:, hh : hh + 1, :],
                            d2v[hh : hh + 1][None, :, :].to_broadcast(
                                (64, 1, QT)
                            ),
                        )
                    norm_state[qn] = (av_sb, rb4)

                def _norm_c(qn):
                    # normalize multiplies ride the Pool engine
                    av_sb, rb4 = norm_state[qn]
                    qs = slice(qn * QT, (qn + 1) * QT)
                    for hh in range(4):
                        nc.gpsimd.tensor_tensor(
                            concatT[64 * (hh % 2) : 64 * (hh % 2) + 64, hh // 2, qs],
                            av_sb[0:64, hh, :],
                            rb4[:, hh, :],
                            MUL,
                        )

                # pair-granular normalize for the last tile: pair0's
                # reciprocal pipeline overlaps pair1's key-chunk loop, so
                # only pair1's DRAM bounce sits in the tail
                half_state = {}

                def _normh_a(qn, pr, eng):
                    _av_sb, d1 = norm_state[qn]
                    s64 = nrm.tile([64, 16], BF16, tag=f"s64_{pr}")
                    eng.dma_start(s64[:], d1[64 * pr : 64 * pr + 64, :])
                    half_state[(qn, pr)] = s64

                def _normh_b(qn, pr, eng):
                    s64 = half_state[(qn, pr)]
                    r64 = nrm.tile([64, 16], BF16, tag=f"r64_{pr}")
                    with nc.allow_low_precision(
                        reason="softmax denominators carry bf16 noise already"
                    ):
                        nc.vector.reciprocal(r64[:], s64[:])
                    d2 = dr.tile([64, 16], BF16, tag=f"dh2_{pr}")
                    eng.dma_start(d2[:], r64[:])
                    if (qn, "rb4") not in half_state:
                        half_state[(qn, "rb4")] = nrm.tile(
                            [64, 4, QT], BF16, tag="rb4", name="rb4h"
                        )
                    rb4 = half_state[(qn, "rb4")]
                    d2v = d2[:].rearrange("p c -> (p c)").rearrange(
                        "(h q) -> h q", h=2
                    )
                    for i, hh in enumerate((2 * pr, 2 * pr + 1)):
                        heng = eng if i % 2 == 0 else nc.sync
                        heng.dma_start(
                            rb4[:, hh : hh + 1, :],
                            d2v[i : i + 1][None, :, :].to_broadcast(
                                (64, 1, QT)
                            ),
                        )

                def _normh_c(qn, pr):
                    av_sb, _d1 = norm_state[qn]
                    rb4 = half_state[(qn, "rb4")]
                    qs = slice(qn * QT, (qn + 1) * QT)
                    for hh in (2 * pr, 2 * pr + 1):
                        nc.gpsimd.tensor_tensor(
                            concatT[64 * (hh % 2) : 64 * (hh % 2) + 64, hh // 2, qs],
                            av_sb[0:64, hh, :],
                            rb4[:, hh, :],
                            MUL,
                        )

                pm_hist = {}
                sc_hist = {}

                def _emit_pm(qt, kc, mbuf, scs):
                    pm = eb.tile([P, 2, QT], BF16, tag="pm", bufs=5)
                    mop = maskf2[:, mbuf, kc, :]
                    if _form(qt, kc) == "B":
                        for h2 in range(2):
                            nc.vector.scalar_tensor_tensor(
                                pm[:, h2, :].bitcast(U16),
                                scs[h2][:],
                                K16,
                                mop.bitcast(BF16),
                                ADD,
                                MIN,
                            )
                    else:
                        ex = eb.tile([P, 2, QT], BF16, tag="ex", bufs=3)
                        for h2 in range(2):
                            nc.scalar.activation(
                                ex[:, h2, :], scs[h2][:], EXP,
                                bias=ebias[:], scale=A_SCALE,
                            )
                        for h2 in range(2):
                            nc.vector.tensor_tensor(
                                pm[:, h2, :].bitcast(U32),
                                ex[:, h2, :].bitcast(U32),
                                mop.bitcast(U32),
                                AND,
                            )
                    pm_hist[kc] = pm

                for qt in range(NQT):
                    qsl = slice(qt * QT, (qt + 1) * QT)
                    mbuf = qt % 2

                    av_sb = nrm.tile([65, 4, QT], BF16, tag="av_sb")
                    d1 = dr.tile([128, 16], BF16, tag="d1")
                    norm_state[qt] = (av_sb, d1)
                    for pair in range(2):
                        avs2 = psACC.tile(
                            [P, 2, QT], F32, tag="acc", bufs=1, name="av2"
                        )

                        def _av(kcd, pm, pair=pair, avs2=avs2):
                            for h2 in range(2):
                                nc.tensor.matmul(
                                    avs2[0 : DH + 1, h2, :],
                                    vp_aug[:, kcd, 2 * pair + h2, :],
                                    pm[:, h2, :],
                                    start=(kcd == 0),
                                    stop=(kcd == NKC - 1),
                                )

                        # software-pipelined: AV for chunk kc-2 is emitted
                        # after the scores of chunk kc, so the exp+mask of a
                        # chunk has two chunk-times to land before the PE
                        # needs its pm
                        for kc in range(NKC):
                            # per-h2 single-bank score tiles: 4-deep psS so
                            # the PE runs 2 chunks ahead of the exp stream
                            scs = []
                            for h2 in range(2):
                                lo = 64 * h2
                                sch = psS.tile([P, QT], F32, tag="sc")
                                nc.tensor.matmul(
                                    sch[:],
                                    kpT[lo : lo + 64, pair, kc * P : (kc + 1) * P],
                                    qpT[lo : lo + 64, pair, qsl],
                                )
                                scs.append(sch)
                            if qt == 0 and pair == 0:
                                # k/v projections ride the first pass
                                for f in {
                                    0: [lambda: _kproj_dma(1)],
                                    1: [lambda: _vx_dma(1),
                                        lambda: _kproj_mm(0, 1)],
                                    2: [lambda: _kproj_mm(1, 0)],
                                    3: [lambda: _kproj_dma(2),
                                        lambda: _qproj_mm(0, 1)],
                                    4: [lambda: _kproj_mm(1, 1)],
                                    5: [lambda: _vx_dma(2)],
                                    6: [lambda: _kproj_mm(2, 0)],
                                    7: [lambda: _kproj_dma(3),
                                        lambda: _kproj_mm(2, 1)],
                                    9: [lambda: _vx_dma(3)],
                                    10: [lambda: _kproj_mm(3, 0)],
                                    11: [lambda: _kproj_mm(3, 1)],
                                }.get(kc, []):
                                    f()
                                v_t = vx_tiles[kc // 4]
                                ps = psA.tile([P, GD], F32, tag="proj", name="psv")
                                for dc in range(NDC):
                                    nc.tensor.matmul(
                                        ps[:],
                                        v_t[:, dc, (kc % 4) * P : (kc % 4 + 1) * P],
                                        wv_s[:, dc, :],
                                        start=(dc == 0),
                                        stop=(dc == NDC - 1),
                                    )
                                if zero_bias:
                                    nc.scalar.copy(
                                        vp_aug[:, kc, :, 0:DH],
                                        ps[:].rearrange(
                                            "p (h d) -> p h d", h=GH
                                        ),
                                    )
                                else:
                                    nc.vector.tensor_tensor(
                                        vp_aug[:, kc, :, 0:DH],
                                        ps[:].rearrange(
                                            "p (h d) -> p h d", h=GH
                                        ),
                                        bv_b[:].rearrange(
                                            "p (h d) -> p h d", h=GH
                                        ),
                                        ADD,
                                    )
                            if pair == 0 and qt + 1 < NQT and (
                                kc == (12 if qt == 0 else 0)
                            ):
                                _qproj_dma(qt + 1)
                            if pair == 0 and kc == 8 and qt + 1 < NQT:
                                # next tile's mask prefetch (deferred off the
                                # critical input head)
                                nsl = slice((qt + 1) * QT, (qt + 2) * QT)
                                nc.gpsimd.dma_start(
                                    maskf2[:, (qt + 1) % 2, :, :],
                                    maskT[:, nsl].rearrange(
                                        "(c p) t -> p c t", p=P
                                    ),
                                )
                            if pair == 0 and qt > 0:
                                if kc == 0:
                                    _norm_a(qt - 1)
                                if kc == 8:
                                    _norm_b(qt - 1)
                                if kc == 5 and qt + 1 < NQT:
                                    _qproj_mm(qt + 1, 0, half=0)
                                if kc == 7 and qt + 1 < NQT:
                                    _qproj_mm(qt + 1, 0, half=1)
                                if kc == 10 and qt + 1 < NQT:
                                    _qproj_mm(qt + 1, 1, half=0)
                                if kc == 12 and qt + 1 < NQT:
                                    _qproj_mm(qt + 1, 1, half=1)
                            if pair == 1:
                                if qt == 0:
                                    if kc == 0:
                                        nc.sync.dma_start(
                                            woT_s[:],
                                            woT[:].rearrange(
                                                "p (c n) -> p c n", c=2
                                            ),
                                        )
                                    if kc == 2:
                                        _qproj_mm(1, 0, half=0)
                                    if kc == 3:
                                        _qproj_mm(1, 0, half=1)
                                    if kc == 5:
                                        _qproj_mm(1, 1, half=0)
                                    if kc == 6:
                                        _qproj_mm(1, 1, half=1)
                                if kc == 0 and qt > 0:
                                    _norm_c(qt - 1)
                                if qt == NQT - 1:
                                    if kc == 3:
                                        _normh_a(qt, 0, nc.scalar)
                                    if kc == 9:
                                        _normh_b(qt, 0, nc.scalar)
                                    if kc == 14:
                                        _normh_c(qt, 0)
                                if qt > 0 and kc in (2, 4, 6, 8, 9, 10, 12, 13):
                                    ih = (2, 4, 6, 8, 9, 10, 12, 13).index(kc)
                                    _cproj_tt(
                                        4 * (qt - 1) + ih // 2, False,
                                        nh_only=ih % 2,
                                    )
                            sc_hist[kc] = scs
                            # batched emission every second chunk: the PE
                            # stream groups [scores x2][AV x2] so same-type
                            # matmuls stay back-to-back, then the exp+mask
                            # ops for both chunks follow
                            if kc % 2 == 1:
                                for kd in (kc - 3, kc - 2):
                                    if kd >= 0:
                                        _av(kd, pm_hist.pop(kd))
                                for kd in (kc - 1, kc):
                                    _emit_pm(qt, kd, mbuf, sc_hist.pop(kd))
                        _av(NKC - 2, pm_hist.pop(NKC - 2))
                        _av(NKC - 1, pm_hist.pop(NKC - 1))
                        # drain av (incl. its sum row 64) out of PSUM on the
                        # ACT engine (its exp load dropped with the split)
                        d1v = d1[:].rearrange("p c -> (p c)").rearrange(
                            "(h q) -> h q", h=4
                        )
                        xeng = (
                            nc.scalar
                            if (qt == NQT - 1 and pair == 1)
                            else nc.sync
                        )
                        nc.scalar.copy(
                            av_sb[:, 2 * pair : 2 * pair + 2, :],
                            avs2[0 : DH + 1, :, :],
                        )
                        for h2 in range(2):
                            hh = 2 * pair + h2
                            xeng.dma_start(
                                d1v[hh : hh + 1, :], av_sb[64:65, hh, :]
                            )
                # tail: only pair1's half-normalize remains; short junk-MM
                # bridges keep the PE clock warm across its DMA bounces
                junkps = psACC.tile(
                    [P, 2, QT], F32, tag="acc", bufs=1, name="junkps"
                )

                def _junk(n):
                    for wi in range(n):
                        nc.tensor.matmul(
                            junkps[:, 0, 0:GD],
                            wq_s[:, wi % NDC, 0:P],
                            wq_s[:, (wi + 1) % NDC, :],
                            start=True,
                            stop=True,
                        )

                _normh_a(NQT - 1, 1, nc.scalar)
                _junk(16)
                _normh_b(NQT - 1, 1, nc.scalar)
                _junk(58)
                _normh_c(NQT - 1, 1)
                _cproj(NQT - 1, last=True)

    _split_excess_waits(nc)
    return nc


_NC_CACHE = {}
LAST_RESULTS = None  # test harness reads exec_time_ns off this


def kernel(q, k, v, mask, Wq, bq, Wk, bk, Wv, bv, Wo, bo):
    global LAST_RESULTS
    zb = not (
        np.any(np.asarray(bq)) or np.any(np.asarray(bk))
        or np.any(np.asarray(bv))
    )
    if zb not in _NC_CACHE:
        _NC_CACHE[zb] = _build_nc(zb)
    _NC = _NC_CACHE[zb]

    q = np.asarray(q, np.float32)
    k = np.asarray(k, np.float32)
    v = np.asarray(v, np.float32)

    bf = ml_dtypes.bfloat16
    qTb = [np.ascontiguousarray(q[b].T.astype(bf)) for b in range(B)]
    kTb = [np.ascontiguousarray(k[b].T.astype(bf)) for b in range(B)]
    vTb = [np.ascontiguousarray(v[b].T.astype(bf)) for b in range(B)]

    # mask u16: per-(kc, qt) block form matching the chunk's exp path
    m_keys_q = np.asarray(mask)[0, 0].T != 0   # [keys, q]
    maskT_u16 = np.empty((S, S), np.uint16)
    for kc in range(NKC):
        rows = slice(kc * P, (kc + 1) * P)
        for qt in range(NQT):
            cols = slice(qt * QT, (qt + 1) * QT)
            blk = m_keys_q[rows, cols]
            fm = _form(qt, kc)
            if fm == "B":
                maskT_u16[rows, cols] = np.where(blk, MKEEP_B, MMASK_B)
            elif fm == "G":
                maskT_u16[rows, cols] = np.where(blk, MKEEP_G, MMASK_G)
            else:
                maskT_u16[rows, cols] = np.where(
                    blk, np.uint16(0xFFFF), np.uint16(0)
                )

    Wq_ = np.asarray(Wq, np.float32) * ALPHA
    Wk_ = np.asarray(Wk, np.float32) * ALPHA
    Wv_ = np.asarray(Wv, np.float32)
    Wo_ = np.asarray(Wo, np.float32)

    def _warr(wT):  # [D, GD] -> [P, NDC*GD] per-partition-contiguous, bf16
        return np.ascontiguousarray(
            wT.reshape(NDC, P, GD)
            .transpose(1, 0, 2)
            .reshape(P, NDC * GD)
            .astype(bf)
        )

    in_maps = []
    for c in range(NCORES):
        b, g = divmod(c, NCORES // B)
        rows = slice(GD * g, GD * (g + 1))
        in_maps.append(
            {
                "qT": qTb[b],
                "kT": kTb[b],
                "vT": vTb[b],
                "maskT": maskT_u16,
                "wqT": _warr(Wq_[rows].T),
                "wkT": _warr(Wk_[rows].T),
                "wvT": _warr(Wv_[rows].T),
                "bq": np.ascontiguousarray(
                    np.asarray(bq, np.float32)[rows] * ALPHA
                ),
                "bk": np.ascontiguousarray(
                    np.asarray(bk, np.float32)[rows] * ALPHA
                ),
                "bv": np.ascontiguousarray(np.asarray(bv, np.float32)[rows]),
                "woT": np.ascontiguousarray(
                    Wo_[:, rows].T.reshape(2, P, D)
                    .transpose(1, 0, 2)
                    .reshape(P, 2 * D)
                    .astype(bf)
                ),
            }
        )

    res = run_bass_kernel_spmd(_NC, in_maps, core_ids=list(range(NCORES)))
    LAST_RESULTS = res

    ng = NCORES // B
    out = np.empty((B, S, D), np.float32)
    for b in range(B):
        acc = res.results[b * ng]["y"].astype(np.float32)
        for g in range(1, ng):
            acc += res.results[b * ng + g]["y"].astype(np.float32)
        out[b] = acc + np.asarray(bo, np.float32)
    return out


# revision 30
# speedup vs baseline: 1.0391x; 1.0187x over previous
"""Multi-head attention (B=2, S=2048, D=1024, H=16) on 8 trn2 NeuronCores.

Sharding: core c handles batch c//4 and head-group c%4 (4 heads, dh'=256
slice of the projection dims).  Each core computes its heads' Q/K/V
projections, transposed-layout attention (scores as [keys, q] so softmax
runs one pass per key chunk and A@V contracts keys on partitions), and a
partial output projection against its Wo column slice.  The host sums the
4 partials per batch and adds bo.

All data stays bf16 (fp8 was measured at 4-8% output error here: attention
output is a cancellation-heavy average, so per-element quantization error
lands flat on the output instead of averaging away).  On top of the bf16
baseline, the softmax exp+mask work - which saturated the Scalar engine at
~168us - is split across THREE engines per key chunk:
- path A (9/16 chunks): ACT exp -> bf16, then ONE DVE u32 AND against a
  0xFFFF/0x0000 mask;
- path B (5/16 chunks, kc%3==2): ONE DVE scalar_tensor_tensor
  (s + 16128) min maskv -> u16: f32->u16 conversion is RNE+saturating, so
  the u16 IS the bf16 bit pattern of exp(s)/2 (Schraudolph, 1.9% rms);
  maskv = {19968.0 keep, -1.0 masked (saturates to bits 0 = +0.0)};
- path G (2/16 chunks, kc in {4,12}): ACT exp, then the mask rides the
  otherwise-idle Pool engine as a multiply by a {1.0, 0.0} bf16 mask.
Scores ship pre-scaled by 128/ln2 (qp,kp each carry alpha=4.8045) so both
exp paths read the same PSUM; path A matches path B's +4.07% sawtooth mean
via its activation bias, and the shared scale cancels in the softmax
normalize.  The mask ships as u16 with the per-(kc,qt)-block form chosen
to match the chunk's path.

Other structure (from the tuned bf16 baseline):
- activations ship pre-transposed ([D, S]) so projections contract D on
  partitions with zero on-chip transposes;
- scores/AV run per head with K=64; two heads of a pair sit at SBUF
  partitions 0-63/64-127 so their matmuls row-pack into the PE;
- row sums come from a ones-column appended to V; they bounce through a
  DRAM scratch to a [128, 16] layout so the reciprocal runs at full
  partition parallelism, then broadcast back with a second DRAM hop;
- the normalize multiplies run on the Pool engine; the A@V PSUM drains and
  half the out-projection drains run on the ACT engine (its exp load
  dropped ~35% from the split) and the rest on the DVE;
- emission order is the schedule: one software-pipelined pass per query
  tile; AV matmuls trail their scores by one key chunk; k/v projections
  (first tile), next tile's q-projection, previous tile's normalization
  and out-projection are spliced into the key-chunk loops in <=1us pieces;
- DMA: the ~3MB critical head (wk/kx/wq/qx) is balanced ~1MB per queue
  across sync/scalar/gpsimd, the first mask arrives in per-4-chunk
  quarters, wo/bias loads are deferred into the loop.
"""

import os
import sys

for _p in ("/opt/trn_rl_repo",):
    if _p not in sys.path and os.path.isdir(_p):
        sys.path.insert(0, _p)

import ml_dtypes
import numpy as np

import concourse.bass as bass
import concourse.mybir as mybir
import concourse.tile as tile
from concourse.vector_clock import ScopedClock
from concourse.bass_utils import run_bass_kernel_spmd


def _ensure_axon_hooks_stub():
    """bass_utils imports antenv.axon_hooks when BASS_TRACE=1 under axon;
    this image lacks the module.  Provide a no-hook stub (tracing is then
    skipped gracefully) unless a real one is already installed."""
    try:
        import antenv.axon_hooks  # noqa: F401
    except ImportError:
        import types

        import antenv

        mod = types.ModuleType("antenv.axon_hooks")
        mod._hook = None
        mod.set_axon_ntff_profile_hook = lambda h: setattr(mod, "_hook", h)
        mod.get_axon_ntff_profile_hook = lambda: mod._hook
        sys.modules["antenv.axon_hooks"] = mod
        antenv.axon_hooks = mod


_ensure_axon_hooks_stub()

F32 = mybir.dt.float32
BF16 = mybir.dt.bfloat16
U8 = mybir.dt.uint8
U16 = mybir.dt.uint16
U32 = mybir.dt.uint32
EXP = mybir.ActivationFunctionType.Exp
MUL = mybir.AluOpType.mult
ADD = mybir.AluOpType.add
MIN = mybir.AluOpType.min
AND = mybir.AluOpType.bitwise_and

B, S, D, H, DH = 2, 2048, 1024, 16, 64
NCORES = 8
GH = 4            # heads per core
GD = GH * DH      # 256, dh' slice per core
P = 128
NDC = D // P      # 8 contraction chunks
NQT = 4           # 512-wide query tiles
QT = 512
NKC = S // P      # 16 key chunks
NTT = S // P      # 16 token tiles

# softmax scaling: qp,kp each carry ALPHA so the score PSUM equals
# s_true * 128/ln2 = bf16-bits-per-e-fold
ALPHA = 4.8044896
K16 = 16128.0                  # path-B bits bias: pm = exp(s)/2
A_SCALE = 0.0054152123         # ln2/128
A_BIAS = -0.6532618            # -ln2 + ln(1.0407) sawtooth-mean match
MKEEP_B = 0x469C               # bf16 19968.0
MMASK_B = 0xBF80               # bf16 -1.0
MKEEP_G = 0x3F80               # bf16 1.0
MMASK_G = 0x0000


def _form(qt: int, kc: int) -> str:
    """exp+mask path per key chunk (must not depend on pair: the mask
    block form is shared between pairs).  Half the chunks ride path B:
    per-chunk engine cost is 1.38us (DVE only) vs 2.39us for path A
    (ACT exp + DVE AND), so this split leaves the ACT stream ~35%
    loaded and exps fire the moment their scores land."""
    return "B" if kc % 2 == 0 else "A"


# ---------------------------------------------------------------------------
# Walrus-compat shims: this neuronxcc build encodes at most ONE sync wait per
# instruction; Tile's wait assigner emits more.  Hoist overflow waits onto
# injected same-engine NOPs placed immediately before the instruction.
# ---------------------------------------------------------------------------
class _TC(tile.TileContext):
    def _drain_and_barrier(self, tick_clock, wait_clock):
        carrier = self.nc.sync.nop(nofuse=True, hint="tail_waits")
        wait_clock.add_sem_waits(
            carrier.ins, ScopedClock({None: tick_clock.global_clock})
        )
        si = carrier.ins.sync_info
        evs = list(si.on_wait) if si is not None else []
        carrier.ins.sync_info = mybir.SyncInfo(on_wait=evs[:1], on_update=[])
        for k in range(1, len(evs)):
            w = self.nc.sync.nop(nofuse=True, hint=f"tail_wait_{k}")
            w.ins.sync_info = mybir.SyncInfo(on_wait=[evs[k]], on_update=[])
        self.nc.sync.drain()
        self.nc.all_engine_barrier()
        assert self.sems is not None
        popped = self.nc._tile_sem_poison_stack.pop()
        assert popped is self._sem_poison
        self.nc.clear_and_free_semaphores(list(self.sems.allocated().values()))
        self.nc.all_engine_barrier()


def _split_excess_waits(nc: bass.Bass) -> int:
    n_split = 0
    uid = 0
    for f in nc.m.functions:
        for bb in f.blocks:
            new_insts = []
            for inst in bb.instructions:
                si = inst.sync_info
                waits = list(si.on_wait) if si is not None else []
                if len(waits) > 1:
                    for ev in waits[:-1]:
                        nop = mybir.InstNoOp(
                            name=f"I-waitsplit-{uid}", ins=[], outs=[]
                        )
                        uid += 1
                        nop.engine = inst.engine
                        nop.bass_nofuse = True
                        nop.sync_info = mybir.SyncInfo(
                            on_wait=[ev], on_update=[]
                        )
                        new_insts.append(nop)
                        n_split += 1
                    inst.sync_info = mybir.SyncInfo(
                        on_wait=waits[-1:], on_update=list(si.on_update)
                    )
                new_insts.append(inst)
            bb.instructions = new_insts
    return n_split


# ---------------------------------------------------------------------------
# Device kernel (identical on all 8 cores; only the input data differs)
# ---------------------------------------------------------------------------
def _build_nc(zero_bias: bool) -> bass.Bass:
    nc = bass.Bass("TRN2", target_bir_lowering=False)

    qT = nc.dram_tensor("qT", [D, S], BF16, kind="ExternalInput")
    kT = nc.dram_tensor("kT", [D, S], BF16, kind="ExternalInput")
    vT = nc.dram_tensor("vT", [D, S], BF16, kind="ExternalInput")
    # mask u16, per-(kc,qt)-block form matching the chunk's exp path
    maskT = nc.dram_tensor("maskT", [S, S], U16, kind="ExternalInput")
    # weights pre-arranged on the host to [P, NDC*GD] / [P, 2*D] lines
    wqT = nc.dram_tensor("wqT", [P, NDC * GD], BF16, kind="ExternalInput")
    wkT = nc.dram_tensor("wkT", [P, NDC * GD], BF16, kind="ExternalInput")
    wvT = nc.dram_tensor("wvT", [P, NDC * GD], BF16, kind="ExternalInput")
    bq = nc.dram_tensor("bq", [GD], F32, kind="ExternalInput")
    bk = nc.dram_tensor("bk", [GD], F32, kind="ExternalInput")
    bv = nc.dram_tensor("bv", [GD], F32, kind="ExternalInput")
    woT = nc.dram_tensor("woT", [P, 2 * D], BF16, kind="ExternalInput")
    y = nc.dram_tensor("y", [S, D], BF16, kind="ExternalOutput")

    with _TC(nc) as tc:
        with (
            tc.tile_pool(name="persist", bufs=1) as pp,
            tc.tile_pool(name="dram", bufs=4, space="DRAM") as dr,
        ):
            # ---- persistent SBUF state ----
            wq_s = pp.tile([P, NDC, GD], BF16)
            wk_s = pp.tile([P, NDC, GD], BF16)
            wv_s = pp.tile([P, NDC, GD], BF16)
            nc.scalar.dma_start(
                wk_s[:, 0:4, :],
                wkT[:].rearrange("p (c m) -> p c m", c=NDC)[:, 0:4, :],
            )
            bq_s = pp.tile([P, 2], F32)
            bk_s = pp.tile([P, 2], F32)
            nc.sync.dma_start(bk_s[:], bk[:].rearrange("(c p) -> p c", p=P))

            qpT = pp.tile([P, 2, S], BF16)   # [dh' within pair-chunk, pair, tok]
            kpT = pp.tile([P, 2, S], BF16)
            vp_aug = pp.tile([P, NKC, GH, DH + 1], BF16)
            concatT = pp.tile([P, 2, S], BF16)
            maskf2 = pp.tile([P, 2, NKC, QT], U16)
            ebias = pp.tile([P, 1], F32)

            nc.vector.memset(ebias[:], A_BIAS)
            nc.vector.memset(vp_aug[:, :, :, DH], 1.0)

            # touch EXP once immediately so the ~2.7us activation-table load
            # happens during the input-DMA head, not before the first real exp
            warm = pp.tile([1, 2], F32)
            nc.vector.memset(warm[:], 0.0)
            nc.scalar.activation(warm[0:1, 0:1], warm[0:1, 1:2], EXP)

            with (
                tc.tile_pool(name="xa", bufs=2) as xa,
                tc.tile_pool(name="eb", bufs=6) as eb,
                tc.tile_pool(name="nrm", bufs=2) as nrm,
                tc.tile_pool(name="yc", bufs=4) as yc,
                tc.tile_pool(name="psA", bufs=2, space="PSUM") as psA,
                tc.tile_pool(name="psS", bufs=4, space="PSUM") as psS,
                tc.tile_pool(name="psACC", bufs=2, space="PSUM") as psACC,
            ):
                ysb_tiles = {}

                def _cproj_tt(tt, last, nh_only=None):
                    """partial out-projection for token tile tt; emitted in
                    nh-halves (2 matmuls + a PSUM drain each); the drains
                    split between the DVE (nh0) and ACT (nh1) engines"""
                    nhs = (0, 1) if nh_only is None else (nh_only,)
                    if nhs[0] == 0:
                        ysb_tiles[tt] = yc.tile(
                            [P, D], BF16, tag="ysb", name=f"ysb{tt}"
                        )
                    y_sb = ysb_tiles[tt]
                    for nh in nhs:
                        yp = psA.tile([P, QT], F32, tag="proj", name="yp")
                        for pc in range(2):
                            nc.tensor.matmul(
                                yp[:],
                                concatT[:, pc, tt * P : (tt + 1) * P],
                                woT_s[:, pc, nh * QT : (nh + 1) * QT],
                                start=(pc == 0),
                                stop=(pc == 1),
                            )
                        if last and nh == 0:
                            nc.vector.tensor_copy(
                                y_sb[:, nh * QT : (nh + 1) * QT], yp[:]
                            )
                        else:
                            nc.scalar.copy(
                                y_sb[:, nh * QT : (nh + 1) * QT], yp[:]
                            )
                    if nhs[-1] == 1:
                        del ysb_tiles[tt]
                        if last:
                            # tail: spread the final y stores across queues
                            e0, e1 = (
                                (nc.sync, nc.scalar)
                                if tt % 2 == 0
                                else (nc.gpsimd, nc.sync)
                            )
                            rows = slice(tt * P, (tt + 1) * P)
                            e0.dma_start(y[rows, 0:QT], y_sb[:, 0:QT])
                            e1.dma_start(y[rows, QT:D], y_sb[:, QT:D])
                        else:
                            nc.sync.dma_start(
                                y[tt * P : (tt + 1) * P, :], y_sb[:]
                            )

                def _cproj(qn, last):
                    for tt in range(4 * qn, 4 * qn + 4):
                        _cproj_tt(tt, last)

                qx_tiles = {}

                def _qproj_dma(qn, engs=None):
                    qs = slice(qn * QT, (qn + 1) * QT)
                    x_t = xa.tile([P, NDC, QT], BF16, tag="x", name=f"xq{qn}")
                    qsrc = qT[:].rearrange("(c p) t -> p c t", p=P)[:, :, qs]
                    engs = engs or (nc.sync, nc.sync)
                    engs[0].dma_start(x_t[:, 0:4, :], qsrc[:, 0:4, :])
                    engs[1].dma_start(x_t[:, 4:8, :], qsrc[:, 4:8, :])
                    qx_tiles[qn] = x_t

                # the 8-matmul accumulation chains of the q/k projections are
                # emitted in dc-halves so a single insertion into the key-chunk
                # loop never stalls the exp stream by more than ~0.5us
                proj_ps = {}

                def _qproj_mm(qn, pc, half=None):
                    qs = slice(qn * QT, (qn + 1) * QT)
                    x_t = qx_tiles[qn]
                    halves = (0, 1) if half is None else (half,)
                    if halves[0] == 0:
                        ps = psA.tile([P, QT], F32, tag="proj", name="psq")
                        proj_ps[("q", qn, pc)] = ps
                    else:
                        ps = proj_ps.pop(("q", qn, pc))
                    for h in halves:
                        for dc in range(4 * h, 4 * h + 4):
                            nc.tensor.matmul(
                                ps[:],
                                wq_s[:, dc, pc * P : (pc + 1) * P],
                                x_t[:, dc, :],
                                start=(dc == 0),
                                stop=(dc == NDC - 1),
                            )
                    if halves[-1] == 1:
                        if zero_bias:
                            nc.scalar.copy(qpT[:, pc, qs], ps[:])
                        else:
                            nc.vector.tensor_scalar_add(
                                qpT[:, pc, qs], ps[:], bq_s[:, pc : pc + 1]
                            )

                def _qproj(qn):
                    _qproj_dma(qn)
                    _qproj_mm(qn, 0)
                    _qproj_mm(qn, 1)

                kx_tiles = {}

                def _kproj_dma(qn, engs=None):
                    x_t = xa.tile(
                        [P, NDC, QT], BF16, tag="kx", bufs=3, name=f"xk{qn}"
                    )
                    ksrc = kT[:].rearrange("(c p) t -> p c t", p=P)[
                        :, :, qn * QT : (qn + 1) * QT
                    ]
                    engs = engs or (nc.sync, nc.sync)
                    engs[0].dma_start(x_t[:, 0:4, :], ksrc[:, 0:4, :])
                    engs[1].dma_start(x_t[:, 4:8, :], ksrc[:, 4:8, :])
                    kx_tiles[qn] = x_t

                def _kproj_mm(qn, pc, half=None):
                    x_t = kx_tiles[qn]
                    halves = (0, 1) if half is None else (half,)
                    if halves[0] == 0:
                        ps = psA.tile([P, QT], F32, tag="proj", name="psk")
                        proj_ps[("k", qn, pc)] = ps
                    else:
                        ps = proj_ps.pop(("k", qn, pc))
                    for h in halves:
                        for dc in range(4 * h, 4 * h + 4):
                            nc.tensor.matmul(
                                ps[:],
                                wk_s[:, dc, pc * P : (pc + 1) * P],
                                x_t[:, dc, :],
                                start=(dc == 0),
                                stop=(dc == NDC - 1),
                            )
                    if halves[-1] == 1:
                        if zero_bias:
                            nc.scalar.copy(
                                kpT[:, pc, qn * QT : (qn + 1) * QT], ps[:]
                            )
                        else:
                            nc.vector.tensor_scalar_add(
                                kpT[:, pc, qn * QT : (qn + 1) * QT],
                                ps[:],
                                bk_s[:, pc : pc + 1],
                            )

                def _kproj(qn):
                    _kproj_dma(qn)
                    _kproj_mm(qn, 0)
                    _kproj_mm(qn, 1)

                vx_tiles = {}

                def _vx_dma(j, engs=None):
                    v_t = xa.tile(
                        [P, NDC, QT], BF16, tag="vx", bufs=2, name=f"xv{j}"
                    )
                    vsrc = vT[:].rearrange("(c p) t -> p c t", p=P)[
                        :, :, j * QT : (j + 1) * QT
                    ]
                    engs = engs or (nc.sync, nc.sync)
                    engs[0].dma_start(v_t[:, 0:4, :], vsrc[:, 0:4, :])
                    engs[1].dma_start(v_t[:, 4:8, :], vsrc[:, 4:8, :])
                    vx_tiles[j] = v_t

                # head: the ~3MB of critical first inputs (wk/kx/wq/qx) is
                # balanced ~1MB per queue across sync/scalar/gpsimd, followed
                # by the v inputs, with the mask in per-4-chunk quarters
                # critical first inputs balanced across the three DMA issue
                # queues (sync/scalar/gpsimd), ~1MB each, strictly ahead of
                # the v-path and mask bytes
                # queue plan (per-queue order IS arrival order):
                #   sync:   kx[0:4] kx[4:8] wv[0:4] vx0-lo
                #   scalar: wk[0:4](persist) wk[4:8] qx[4:8] wv[4:8] mask...
                #   gpsimd: wq[0:4] qx[0:4] wq[4:8] bq vx0-hi bv mask...
                _kproj_dma(0, engs=(nc.sync, nc.sync))
                wkr = wkT[:].rearrange("p (c m) -> p c m", c=NDC)
                nc.scalar.dma_start(wk_s[:, 4:8, :], wkr[:, 4:8, :])
                wqr = wqT[:].rearrange("p (c m) -> p c m", c=NDC)
                nc.gpsimd.dma_start(wq_s[:, 0:4, :], wqr[:, 0:4, :])
                qs0 = qT[:].rearrange("(c p) t -> p c t", p=P)[:, :, 0:QT]
                x_q0 = xa.tile([P, NDC, QT], BF16, tag="x", name="xq0")
                nc.gpsimd.dma_start(x_q0[:, 0:4, :], qs0[:, 0:4, :])
                nc.scalar.dma_start(x_q0[:, 4:8, :], qs0[:, 4:8, :])
                nc.gpsimd.dma_start(wq_s[:, 4:8, :], wqr[:, 4:8, :])
                qx_tiles[0] = x_q0
                nc.gpsimd.dma_start(bq_s[:], bq[:].rearrange("(c p) -> p c", p=P))
                # warm the PE HAM gate on the first-landed weight quarter
                # while the big x-tiles are still streaming in
                wjunk = psA.tile([P, GD], F32, tag="proj", name="wjunk")
                for wi in range(26):
                    nc.tensor.matmul(
                        wjunk[:],
                        wk_s[:, wi % 4, 0:P],
                        wk_s[:, (wi + 1) % 4, :],
                        start=True,
                        stop=True,
                    )
                _kproj_mm(0, 0, half=0)
                _qproj_mm(0, 0, half=0)
                _kproj_mm(0, 0, half=1)
                _qproj_mm(0, 0, half=1)
                wvr = wvT[:].rearrange("p (c m) -> p c m", c=NDC)
                nc.sync.dma_start(wv_s[:, 0:4, :], wvr[:, 0:4, :])
                nc.scalar.dma_start(wv_s[:, 4:8, :], wvr[:, 4:8, :])
                _vx_dma(0, engs=(nc.sync, nc.gpsimd))
                bv_b = pp.tile([P, GD], F32)
                nc.gpsimd.dma_start(bv_b[:], bv[:][None, :].to_broadcast((P, GD)))
                mr0 = maskT[:, 0:QT].rearrange("(c p) t -> p c t", p=P)
                for mq, meng in zip(
                    range(4), (nc.gpsimd, nc.scalar, nc.gpsimd, nc.scalar)
                ):
                    meng.dma_start(
                        maskf2[:, 0, 4 * mq : 4 * mq + 4, :],
                        mr0[:, 4 * mq : 4 * mq + 4, :],
                    )
                woT_s = pp.tile([P, 2, D], BF16)

                # ---- per query tile: q-proj, attention, partial out-proj --
                norm_state = {}

                def _norm_a(qn, eng=None):
                    av_sb, d1 = norm_state[qn]
                    s128 = nrm.tile([128, 16], BF16, tag="s128")
                    (eng or nc.sync).dma_start(s128[:], d1[:])
                    norm_state[qn] = (av_sb, s128)

                def _norm_b(qn, eng=None):
                    av_sb, s128 = norm_state[qn]
                    r128 = nrm.tile([128, 16], BF16, tag="r128")
                    with nc.allow_low_precision(
                        reason="softmax denominators carry bf16 noise already"
                    ):
                        nc.vector.reciprocal(r128[:], s128[:])
                    d2 = dr.tile([128, 16], BF16, tag="d2")
                    (eng or nc.sync).dma_start(d2[:], r128[:])
                    rb4 = nrm.tile([64, 4, QT], BF16, tag="rb4")
                    d2v = d2[:].rearrange("p c -> (p c)").rearrange(
                        "(h q) -> h q", h=4
                    )
                    for hh in range(4):
                        heng = (eng or nc.sync) if hh % 2 == 0 else nc.sync
                        heng.dma_start(
                            rb4[:, hh : hh + 1, :],
                            d2v[hh : hh + 1][None, :, :].to_broadcast(
                                (64, 1, QT)
                            ),
                        )
                    norm_state[qn] = (av_sb, rb4)

                def _norm_c(qn):
                    # normalize multiplies ride the Pool engine
                    av_sb, rb4 = norm_state[qn]
                    qs = slice(qn * QT, (qn + 1) * QT)
                    for hh in range(4):
                        nc.gpsimd.tensor_tensor(
                            concatT[64 * (hh % 2) : 64 * (hh % 2) + 64, hh // 2, qs],
                            av_sb[0:64, hh, :],
                            rb4[:, hh, :],
                            MUL,
                        )

                # pair-granular normalize for the last tile: pair0's
                # reciprocal pipeline overlaps pair1's key-chunk loop, so
                # only pair1's DRAM bounce sits in the tail
                half_state = {}

                def _normh_a(qn, pr, eng):
                    _av_sb, d1 = norm_state[qn]
                    s64 = nrm.tile([64, 16], BF16, tag=f"s64_{pr}")
                    eng.dma_start(s64[:], d1[64 * pr : 64 * pr + 64, :])
                    half_state[(qn, pr)] = s64

                def _normh_b(qn, pr, eng):
                    s64 = half_state[(qn, pr)]
                    r64 = nrm.tile([64, 16], BF16, tag=f"r64_{pr}")
                    with nc.allow_low_precision(
                        reason="softmax denominators carry bf16 noise already"
                    ):
                        nc.vector.reciprocal(r64[:], s64[:])
                    d2 = dr.tile([64, 16], BF16, tag=f"dh2_{pr}")
                    eng.dma_start(d2[:], r64[:])
                    if (qn, "rb4") not in half_state:
                        half_state[(qn, "rb4")] = nrm.tile(
                            [64, 4, QT], BF16, tag="rb4", name="rb4h"
                        )
                    rb4 = half_state[(qn, "rb4")]
                    d2v = d2[:].rearrange("p c -> (p c)").rearrange(
                        "(h q) -> h q", h=2
                    )
                    for i, hh in enumerate((2 * pr, 2 * pr + 1)):
                        heng = eng if i % 2 == 0 else nc.sync
                        heng.dma_start(
                            rb4[:, hh : hh + 1, :],
                            d2v[i : i + 1][None, :, :].to_broadcast(
                                (64, 1, QT)
                            ),
                        )

                def _normh_c(qn, pr):
                    av_sb, _d1 = norm_state[qn]
                    rb4 = half_state[(qn, "rb4")]
                    qs = slice(qn * QT, (qn + 1) * QT)
                    for hh in (2 * pr, 2 * pr + 1):
                        nc.gpsimd.tensor_tensor(
                            concatT[64 * (hh % 2) : 64 * (hh % 2) + 64, hh // 2, qs],
                            av_sb[0:64, hh, :],
                            rb4[:, hh, :],
                            MUL,
                        )

                pm_hist = {}
                sc_hist = {}

                def _emit_pm(qt, kc, mbuf, scs):
                    pm = eb.tile([P, 2, QT], BF16, tag="pm", bufs=5)
                    mop = maskf2[:, mbuf, kc, :]
                    if _form(qt, kc) == "B":
                        for h2 in range(2):
                            nc.vector.scalar_tensor_tensor(
                                pm[:, h2, :].bitcast(U16),
                                scs[h2][:],
                                K16,
                                mop.bitcast(BF16),
                                ADD,
                                MIN,
                            )
                    else:
                        ex = eb.tile([P, 2, QT], BF16, tag="ex", bufs=3)
                        for h2 in range(2):
                            nc.scalar.activation(
                                ex[:, h2, :], scs[h2][:], EXP,
                                bias=ebias[:], scale=A_SCALE,
                            )
                        for h2 in range(2):
                            nc.vector.tensor_tensor(
                                pm[:, h2, :].bitcast(U32),
                                ex[:, h2, :].bitcast(U32),
                                mop.bitcast(U32),
                                AND,
                            )
                    pm_hist[kc] = pm

                for qt in range(NQT):
                    qsl = slice(qt * QT, (qt + 1) * QT)
                    mbuf = qt % 2

                    av_sb = nrm.tile([65, 4, QT], BF16, tag="av_sb")
                    d1 = dr.tile([128, 16], BF16, tag="d1")
                    norm_state[qt] = (av_sb, d1)
                    for pair in range(2):
                        avs2 = psACC.tile(
                            [P, 2, QT], F32, tag="acc", bufs=1, name="av2"
                        )

                        def _av(kcd, pm, pair=pair, avs2=avs2):
                            for h2 in range(2):
                                nc.tensor.matmul(
                                    avs2[0 : DH + 1, h2, :],
                                    vp_aug[:, kcd, 2 * pair + h2, :],
                                    pm[:, h2, :],
                                    start=(kcd == 0),
                                    stop=(kcd == NKC - 1),
                                )

                        # software-pipelined: AV for chunk kc-2 is emitted
                        # after the scores of chunk kc, so the exp+mask of a
                        # chunk has two chunk-times to land before the PE
                        # needs its pm
                        for kc in range(NKC):
                            # per-h2 single-bank score tiles: 4-deep psS so
                            # the PE runs 2 chunks ahead of the exp stream
                            scs = []
                            for h2 in range(2):
                                lo = 64 * h2
                                sch = psS.tile([P, QT], F32, tag="sc")
                                nc.tensor.matmul(
                                    sch[:],
                                    kpT[lo : lo + 64, pair, kc * P : (kc + 1) * P],
                                    qpT[lo : lo + 64, pair, qsl],
                                )
                                scs.append(sch)
                            if qt == 0 and pair == 0:
                                # k/v projections ride the first pass
                                for f in {
                                    0: [lambda: _kproj_dma(1)],
                                    1: [lambda: _vx_dma(1),
                                        lambda: _kproj_mm(0, 1)],
                                    2: [lambda: _kproj_mm(1, 0)],
                                    3: [lambda: _kproj_dma(2),
                                        lambda: _qproj_mm(0, 1)],
                                    4: [lambda: _kproj_mm(1, 1)],
                                    5: [lambda: _vx_dma(2)],
                                    6: [lambda: _kproj_mm(2, 0)],
                                    7: [lambda: _kproj_dma(3),
                                        lambda: _kproj_mm(2, 1)],
                                    9: [lambda: _vx_dma(3)],
                                    10: [lambda: _kproj_mm(3, 0)],
                                    11: [lambda: _kproj_mm(3, 1)],
                                }.get(kc, []):
                                    f()
                                v_t = vx_tiles[kc // 4]
                                ps = psA.tile([P, GD], F32, tag="proj", name="psv")
                                for dc in range(NDC):
                                    nc.tensor.matmul(
                                        ps[:],
                                        v_t[:, dc, (kc % 4) * P : (kc % 4 + 1) * P],
                                        wv_s[:, dc, :],
                                        start=(dc == 0),
                                        stop=(dc == NDC - 1),
                                    )
                                if zero_bias:
                                    nc.scalar.copy(
                                        vp_aug[:, kc, :, 0:DH],
                                        ps[:].rearrange(
                                            "p (h d) -> p h d", h=GH
                                        ),
                                    )
                                else:
                                    nc.vector.tensor_tensor(
                                        vp_aug[:, kc, :, 0:DH],
                                        ps[:].rearrange(
                                            "p (h d) -> p h d", h=GH
                                        ),
                                        bv_b[:].rearrange(
                                            "p (h d) -> p h d", h=GH
                                        ),
                                        ADD,
                                    )
                            if pair == 0 and qt + 1 < NQT and (
                                kc == (12 if qt == 0 else 0)
                            ):
                                _qproj_dma(qt + 1)
                            if pair == 0 and kc == 8 and qt + 1 < NQT:
                                # next tile's mask prefetch (deferred off the
                                # critical input head)
                                nsl = slice((qt + 1) * QT, (qt + 2) * QT)
                                nc.gpsimd.dma_start(
                                    maskf2[:, (qt + 1) % 2, :, :],
                                    maskT[:, nsl].rearrange(
                                        "(c p) t -> p c t", p=P
                                    ),
                                )
                            if pair == 0 and qt > 0:
                                if kc == 0:
                                    _norm_a(qt - 1)
                                if kc == 8:
                                    _norm_b(qt - 1)
                                if kc == 5 and qt + 1 < NQT:
                                    _qproj_mm(qt + 1, 0, half=0)
                                if kc == 7 and qt + 1 < NQT:
                                    _qproj_mm(qt + 1, 0, half=1)
                                if kc == 10 and qt + 1 < NQT:
                                    _qproj_mm(qt + 1, 1, half=0)
                                if kc == 12 and qt + 1 < NQT:
                                    _qproj_mm(qt + 1, 1, half=1)
                            if pair == 1:
                                if qt == 0:
                                    if kc == 0:
                                        nc.sync.dma_start(
                                            woT_s[:],
                                            woT[:].rearrange(
                                                "p (c n) -> p c n", c=2
                                            ),
                                        )
                                    if kc == 2:
                                        _qproj_mm(1, 0, half=0)
                                    if kc == 3:
                                        _qproj_mm(1, 0, half=1)
                                    if kc == 5:
                                        _qproj_mm(1, 1, half=0)
                                    if kc == 6:
                                        _qproj_mm(1, 1, half=1)
                                if kc == 0 and qt > 0:
                                    _norm_c(qt - 1)
                                if qt == NQT - 1:
                                    if kc == 3:
                                        _normh_a(qt, 0, nc.scalar)
                                    if kc == 9:
                                        _normh_b(qt, 0, nc.scalar)
                                    if kc == 14:
                                        _normh_c(qt, 0)
                                if qt > 0 and kc in (2, 4, 6, 8, 9, 10, 12, 13):
                                    ih = (2, 4, 6, 8, 9, 10, 12, 13).index(kc)
                                    _cproj_tt(
                                        4 * (qt - 1) + ih // 2, False,
                                        nh_only=ih % 2,
                                    )
                            sc_hist[kc] = scs
                            # batched emission every second chunk: the PE
                            # stream groups [scores x2][AV x2] so same-type
                            # matmuls stay back-to-back, then the exp+mask
                            # ops for both chunks follow
                            if kc % 2 == 1:
                                for kd in (kc - 3, kc - 2):
                                    if kd >= 0:
                                        _av(kd, pm_hist.pop(kd))
                                for kd in (kc - 1, kc):
                                    _emit_pm(qt, kd, mbuf, sc_hist.pop(kd))
                        _av(NKC - 2, pm_hist.pop(NKC - 2))
                        _av(NKC - 1, pm_hist.pop(NKC - 1))
                        # drain av (incl. its sum row 64) out of PSUM on the
                        # ACT engine (its exp load dropped with the split)
                        d1v = d1[:].rearrange("p c -> (p c)").rearrange(
                            "(h q) -> h q", h=4
                        )
                        xeng = (
                            nc.scalar
                            if (qt == NQT - 1 and pair == 1)
                            else nc.sync
                        )
                        nc.scalar.copy(
                            av_sb[:, 2 * pair : 2 * pair + 2, :],
                            avs2[0 : DH + 1, :, :],
                        )
                        for h2 in range(2):
                            hh = 2 * pair + h2
                            xeng.dma_start(
                                d1v[hh : hh + 1, :], av_sb[64:65, hh, :]
                            )
                # tail: only pair1's half-normalize remains; short junk-MM
                # bridges keep the PE clock warm across its DMA bounces
                junkps = psACC.tile(
                    [P, 2, QT], F32, tag="acc", bufs=1, name="junkps"
                )

                def _junk(n):
                    for wi in range(n):
                        nc.tensor.matmul(
                            junkps[:, 0, 0:GD],
                            wq_s[:, wi % NDC, 0:P],
                            wq_s[:, (wi + 1) % NDC, :],
                            start=True,
                            stop=True,
                        )

                _normh_a(NQT - 1, 1, nc.scalar)
                _junk(16)
                _normh_b(NQT - 1, 1, nc.scalar)
                _junk(88)
                _normh_c(NQT - 1, 1)
                _cproj(NQT - 1, last=True)

    _split_excess_waits(nc)
    return nc


_NC_CACHE = {}
LAST_RESULTS = None  # test harness reads exec_time_ns off this


def kernel(q, k, v, mask, Wq, bq, Wk, bk, Wv, bv, Wo, bo):
    global LAST_RESULTS
    zb = not (
        np.any(np.asarray(bq)) or np.any(np.asarray(bk))
        or np.any(np.asarray(bv))
    )
    if zb not in _NC_CACHE:
        _NC_CACHE[zb] = _build_nc(zb)
    _NC = _NC_CACHE[zb]

    q = np.asarray(q, np.float32)
    k = np.asarray(k, np.float32)
    v = np.asarray(v, np.float32)

    bf = ml_dtypes.bfloat16
    qTb = [np.ascontiguousarray(q[b].T.astype(bf)) for b in range(B)]
    kTb = [np.ascontiguousarray(k[b].T.astype(bf)) for b in range(B)]
    vTb = [np.ascontiguousarray(v[b].T.astype(bf)) for b in range(B)]

    # mask u16: per-(kc, qt) block form matching the chunk's exp path
    m_keys_q = np.asarray(mask)[0, 0].T != 0   # [keys, q]
    maskT_u16 = np.empty((S, S), np.uint16)
    for kc in range(NKC):
        rows = slice(kc * P, (kc + 1) * P)
        for qt in range(NQT):
            cols = slice(qt * QT, (qt + 1) * QT)
            blk = m_keys_q[rows, cols]
            fm = _form(qt, kc)
            if fm == "B":
                maskT_u16[rows, cols] = np.where(blk, MKEEP_B, MMASK_B)
            elif fm == "G":
                maskT_u16[rows, cols] = np.where(blk, MKEEP_G, MMASK_G)
            else:
                maskT_u16[rows, cols] = np.where(
                    blk, np.uint16(0xFFFF), np.uint16(0)
                )

    Wq_ = np.asarray(Wq, np.float32) * ALPHA
    Wk_ = np.asarray(Wk, np.float32) * ALPHA
    Wv_ = np.asarray(Wv, np.float32)
    Wo_ = np.asarray(Wo, np.float32)

    def _warr(wT):  # [D, GD] -> [P, NDC*GD] per-partition-contiguous, bf16
        return np.ascontiguousarray(
            wT.reshape(NDC, P, GD)
            .transpose(1, 0, 2)
            .reshape(P, NDC * GD)
            .astype(bf)
        )

    in_maps = []
    for c in range(NCORES):
        b, g = divmod(c, NCORES // B)
        rows = slice(GD * g, GD * (g + 1))
        in_maps.append(
            {
                "qT": qTb[b],
                "kT": kTb[b],
                "vT": vTb[b],
                "maskT": maskT_u16,
                "wqT": _warr(Wq_[rows].T),
                "wkT": _warr(Wk_[rows].T),
                "wvT": _warr(Wv_[rows].T),
                "bq": np.ascontiguousarray(
                    np.asarray(bq, np.float32)[rows] * ALPHA
                ),
                "bk": np.ascontiguousarray(
                    np.asarray(bk, np.float32)[rows] * ALPHA
                ),
                "bv": np.ascontiguousarray(np.asarray(bv, np.float32)[rows]),
                "woT": np.ascontiguousarray(
                    Wo_[:, rows].T.reshape(2, P, D)
                    .transpose(1, 0, 2)
                    .reshape(P, 2 * D)
                    .astype(bf)
                ),
            }
        )

    res = run_bass_kernel_spmd(_NC, in_maps, core_ids=list(range(NCORES)))
    LAST_RESULTS = res

    ng = NCORES // B
    out = np.empty((B, S, D), np.float32)
    for b in range(B):
        acc = res.results[b * ng]["y"].astype(np.float32)
        for g in range(1, ng):
            acc += res.results[b * ng + g]["y"].astype(np.float32)
        out[b] = acc + np.asarray(bo, np.float32)
    return out
